# revision 74
# baseline (speedup 1.0000x reference)
"""GAT layer (nn_GATLayer) on 8 Trainium2 NeuronCores.

Math (reference):
    Wh = X @ weight                      [N, F]
    s  = Wh @ a[:F];  t = Wh @ a[F:]     [N, 1]
    e  = relu(s_i + t_j)                 [N, N]
    att = softmax(where(A > 0, e, -9e15), axis=1)
    out = elu(att @ Wh)

Kernel formulation (shift-free softmax, exact up to fp rounding):
    p_ij  = A_ij * max(exp(s_i) * exp(t_j), 1)   (exp(relu(x)) = max(exp(x), 1))
    out_i = elu((p_i: @ Wh) / sum_j p_ij)
A global scale c (=1/4) keeps all fp8 operands in e4m3 normal range:
the host mask carries {0, c}, z' = exp(s - ln(1/c)) * exp(t), and every
psum contribution is uniformly c^2-scaled, which cancels in num/den.

Sharding: 1D row partition across 8 cores (1024 rows each). Host-side
prep is layout/dtype only: X^T (bf16, grouped, rotated so group 0 is
own rows), A^T slab per core (fp8 {0, c}, same group rotation), weight
bf16. All model math (Wh, s, t, exp, softmax, aggregation, ELU) runs
on device.

Per-core dataflow, all in [j (partition), i (free)] orientation:
  - setup: w_all[k] = [weight_k | w*a2 | w*a1]; per j-tile pair one PE
    pass gives [Wh | t] (fp8 Wh); s from group-0 X^T; es/s broadcast
    rows via K=1 PE outer products; t transposed to a row for exp.
  - main loop over 32 j-tile pairs, three z sources balanced across
    engines (PE rank-1 outer product into bf16 psum / DVE 4x ptr-mult
    / ACT exp with bias ptr); two mask paths:
      D-pairs: p8 = (c max z) * mask in one DVE op -> 2 DoubleRow
        matmuls (numerator [128 f, 1024 i], denominator [1, 1024]);
      P-pairs: mask-term DR matmuls consume the raw fp8 mask, the
        relu-term r8 = Pool tensor_tensor mult of relu(z - c).
  - epilogue: den -> columns via K=1 matmuls, reciprocal, PE
    transposes to natural [i, f], fused scale+ELU, one output DMA.
"""

import numpy as np
import ml_dtypes

import concourse.bass as bass
import concourse.bacc as bacc
import concourse.mybir as mybir
import concourse.tile as tile
from concourse.bass_utils import run_bass_kernel_spmd

N = 8192
F_IN = 512
F_OUT = 128
N_CORES = 8
ROWS = N // N_CORES          # 1024 rows per core
NT = N // 128                # 64 j tiles
NP = NT // 2                 # 32 j tile pairs
KC = F_IN // 128             # 4 f_in chunks
NG = 8                       # XT groups (8 j-tiles each)

SCALE = 0.25                 # global fp8 range scale c
LNS = float(np.log(1.0 / SCALE))

FP32 = mybir.dt.float32
BF16 = mybir.dt.bfloat16
FP8 = mybir.dt.float8e4
Alu = mybir.AluOpType
Act = mybir.ActivationFunctionType
DR = mybir.MatmulPerfMode.DoubleRow

# --- engine lane tables (tuned against the TimelineSim cost model) ---
# P_PAIRS: pairs routed through the relu decomposition (Pool mask-mult)
P_PAIRS = frozenset({1, 3, 5, 6, 9, 11, 13, 14, 17, 19, 21, 22, 25, 26})
# z source per pair: 'pe' (rank-1 matmul into psum, D-pairs only),
# 'act' (exp), 'dve' (ptr-mult)
_ZSRC = {}
for _t in range(NP):
    _ZSRC[_t] = "dve" if _t in P_PAIRS else "act"
# relu engine for P-pairs: DVE 4x for most, ACT for some
_RELU_ACT = frozenset({3, 11, 19, 27})

_cache = {}


def _build():
    nc = bacc.Bacc("TRN2", target_bir_lowering=False, debug=False,
                   num_devices=N_CORES)

    XTg = nc.dram_tensor("XTg", [NG, KC, 128, ROWS], BF16, kind="ExternalInput")
    AT8 = nc.dram_tensor("AT8", [N, ROWS], FP8, kind="ExternalInput")
    w_in = nc.dram_tensor("w_in", [F_IN, F_OUT], BF16, kind="ExternalInput")
    a_vec = nc.dram_tensor("a_vec", [2 * F_OUT, 1], FP32, kind="ExternalInput")
    ident = nc.dram_tensor("ident", [128, 128], FP32, kind="ExternalInput")
    out_d = nc.dram_tensor("out", [ROWS, F_OUT], FP32, kind="ExternalOutput")

    with tile.TileContext(nc) as tc:
        _body(nc, tc, XTg, AT8, w_in, a_vec, ident, out_d)

    nc.compile()
    return nc


def _body(nc, tc, XTg, AT8, w_in, a_vec, ident, out_d):
    with (
        tc.tile_pool(name="setup", bufs=1) as setup,
        tc.tile_pool(name="xtg", bufs=2) as xtg_pool,
        tc.tile_pool(name="at", bufs=1) as at_pool,
        tc.tile_pool(name="zz", bufs=5) as zz_pool,
        tc.tile_pool(name="pp", bufs=6) as pp_pool,
        tc.tile_pool(name="epi", bufs=1) as epi,
    ):
        setup_psum = tc.tile_pool(name="psA", bufs=1, space="PSUM")
        psA = setup_psum.__enter__()
        # ---------------- setup: weights ----------------
        w_sb = setup.tile([128, KC, 128], BF16)
        nc.sync.dma_start(
            out=w_sb, in_=w_in.rearrange("(k p) f -> p k f", p=128)
        )
        idn = setup.tile([128, 128], FP32)
        nc.sync.dma_start(out=idn, in_=ident[:, :])
        a_cat = setup.tile([128, 2], BF16)
        nc.gpsimd.dma_start(
            out=a_cat, in_=a_vec.rearrange("(h p) o -> p (h o)", p=128)
        )

        # first XT group doubles as own-rows X^T (host rotates groups)
        xtg0 = xtg_pool.tile([128, KC, ROWS], BF16, tag="g0")
        nc.sync.dma_start(out=xtg0, in_=XTg[0].rearrange("k p i -> p k i"))
        idn_bf = setup.tile([128, 128], BF16)
        nc.vector.tensor_copy(idn_bf, idn)

        # w_all[k] = [weight_k | w_t_k | w_s_k]  [128, 130]
        w_all = []
        for k in range(KC):
            wa = setup.tile([128, F_OUT + 2], BF16, tag=f"w_all{k}")
            nc.vector.tensor_copy(wa[:, 0:F_OUT], w_sb[:, k, :])
            ps_wT = psA.tile([128, 128], BF16, tag="wT")
            nc.tensor.transpose(ps_wT, w_sb[:, k, :], idn_bf)
            wT = setup.tile([128, 128], BF16, tag=f"wT{k}")
            nc.vector.tensor_copy(wT, ps_wT)
            ps_a = psA.tile([128, 2], FP32, tag="pa")
            nc.tensor.matmul(ps_a, wT, a_cat, start=True, stop=True)
            # col F_OUT = w_t (a[F:]), col F_OUT+1 = w_s (a[:F])
            nc.vector.tensor_copy(wa[:, F_OUT : F_OUT + 1], ps_a[:, 1:2])
            nc.vector.tensor_copy(wa[:, F_OUT + 1 : F_OUT + 2], ps_a[:, 0:1])
            w_all.append(wa)


        # ---------------- s (own rows) + broadcast rows ----------------
        ps_s = psA.tile([1, ROWS], FP32, tag="ps_s")
        for h in range(2):
            sl = slice(512 * h, 512 * (h + 1))
            for k in range(KC):
                nc.tensor.matmul(
                    ps_s[:, sl],
                    w_all[k][:, F_OUT + 1 : F_OUT + 2],
                    xtg0[:, k, sl],
                    start=(k == 0), stop=(k == KC - 1),
                    skip_group_check=True,
                )
        nls1 = setup.tile([1, 1], FP32, tag="nls1")
        nc.vector.memset(nls1, -LNS)
        es_row = setup.tile([1, ROWS], BF16)
        nc.scalar.activation(out=es_row, in_=ps_s, func=Act.Exp, bias=nls1)
        s_row = setup.tile([1, ROWS], BF16)
        nc.scalar.copy(s_row, ps_s)

        ones_r = setup.tile([1, 128], BF16)
        nc.vector.memset(ones_r, 1.0)
        es_bc = setup.tile([128, ROWS], BF16)
        s_bc = setup.tile([128, ROWS], BF16)
        for h in range(2):
            sl = slice(512 * h, 512 * (h + 1))
            ps_b = psA.tile([128, 512], FP32, tag="bc", bufs=2)
            nc.tensor.matmul(ps_b, ones_r, es_row[:, sl], start=True, stop=True)
            nc.vector.tensor_copy(es_bc[:, sl], ps_b)
            ps_b2 = psA.tile([128, 512], FP32, tag="bc", bufs=2)
            nc.tensor.matmul(ps_b2, ones_r, s_row[:, sl], start=True, stop=True)
            nc.vector.tensor_copy(s_bc[:, sl], ps_b2)

        ones_c = setup.tile([128, 2, 128], FP8)
        nc.vector.memset(ones_c, 1.0)
        ones1 = setup.tile([1, 1], FP32, tag="ones1")
        nc.vector.memset(ones1, 1.0)

        # ---------------- Wh | t for all j tiles ----------------
        wh_all = setup.tile([128, NT, F_OUT], FP8)
        et_cols = setup.tile([128, NT], FP32)
        ts_cols = setup.tile([128, NT], FP32)
        nsc_c = setup.tile([128, 1], FP32, tag="nsc_c")
        nc.vector.memset(nsc_c, -SCALE)
        nsc_ln = setup.tile([128, 1], FP32, tag="nsc_ln")
        nc.vector.memset(nsc_ln, -LNS)

        setup_psum.__exit__(None, None, None)

        with (
            tc.tile_pool(name="psO", bufs=1, space="PSUM") as psO,
            tc.tile_pool(name="psD", bufs=1, space="PSUM") as psD,
        ):
            ps_oT = psO.tile([128, ROWS], FP32)
            ps_d = psD.tile([128, ROWS], FP32)

            with tc.tile_pool(name="psS", bufs=4, space="PSUM") as psS:
                at_tiles = []

                def emit_setup(g):
                    if g == 0:
                        xtg = xtg0
                    else:
                        xtg = xtg_pool.tile([128, KC, ROWS], BF16)
                        nc.sync.dma_start(
                            out=xtg, in_=XTg[g].rearrange("k p i -> p k i")
                        )
                    at = at_pool.tile([128, 8, ROWS], FP8, tag=f"at{g}")
                    at_tiles.append(at)
                    nc.sync.dma_start(
                        out=at,
                        in_=AT8[ROWS * g : ROWS * (g + 1), :].rearrange(
                            "(t p) i -> p t i", p=128
                        ),
                    )
                    for q in range(4):
                        jt0 = 8 * g + 2 * q
                        ps_p = psS.tile([128, 2, F_OUT + 2], FP32)
                        for v in range(2):
                            co = 128 * (2 * q + v)
                            for k in range(KC):
                                nc.tensor.matmul(
                                    ps_p[:, v, :],
                                    xtg[:, k, co : co + 128],
                                    w_all[k],
                                    start=(k == 0), stop=(k == KC - 1),
                                    skip_group_check=True,
                                )
                        # copies (gpsimd cannot access PSUM)
                        nc.scalar.copy(
                            wh_all[:, jt0 : jt0 + 2, :], ps_p[:, :, 0:F_OUT]
                        )
                        nc.scalar.activation(
                            out=et_cols[:, jt0 : jt0 + 2],
                            in_=ps_p[:, :, F_OUT : F_OUT + 1],
                            func=Act.Exp,
                        )
                        nc.vector.tensor_scalar(
                            out=ts_cols[:, jt0 : jt0 + 2],
                            in0=ps_p[:, :, F_OUT : F_OUT + 1],
                            scalar1=-LNS, scalar2=None, op0=Alu.add,
                        )

                z_early = {}

                def emit_z(t, zp):
                    for v in range(2):
                        jt = 2 * t + v
                        if _ZSRC[t] == "act":
                            nc.scalar.activation(
                                out=zp[:, v, :], in_=s_bc, func=Act.Exp,
                                bias=ts_cols[:, jt : jt + 1],
                            )
                        else:
                            nc.vector.tensor_scalar(
                                out=zp[:, v, :], in0=es_bc,
                                scalar1=et_cols[:, jt : jt + 1],
                                scalar2=None, op0=Alu.mult,
                            )

                def emit_pair(t):
                    at = at_tiles[t // 4]
                    s0 = 2 * (t % 4)
                    atsl = at[:, s0 : s0 + 2, :]
                    first, last = t == 0, t == NP - 1
                    if t in z_early:
                        zp = z_early[t]
                    else:
                        zp = zz_pool.tile([128, 2, ROWS], BF16)
                        emit_z(t, zp)
                    if t in P_PAIRS:
                        # clamp on DVE (4x), mask-mult on Pool: the host
                        # mask already carries the scale c
                        rt = pp_pool.tile([128, 2, ROWS], BF16, tag="rt")
                        nc.vector.tensor_scalar(
                            out=rt, in0=zp, scalar1=SCALE, scalar2=None,
                            op0=Alu.max,
                        )
                        pp = pp_pool.tile([128, 2, ROWS], FP8, tag="p8")
                        nc.gpsimd.tensor_tensor(
                            out=pp, in0=rt, in1=atsl, op=Alu.mult
                        )
                    else:
                        pp = pp_pool.tile([128, 2, ROWS], FP8, tag="p8")
                        nc.vector.scalar_tensor_tensor(
                            out=pp, in0=zp, scalar=SCALE,
                            in1=atsl, op0=Alu.max, op1=Alu.mult,
                        )
                    if True:
                        for h in range(2):
                            sl = slice(512 * h, 512 * (h + 1))
                            nc.tensor.matmul(
                                ps_oT[:, sl],
                                wh_all[:, 2 * t : 2 * t + 2, :],
                                pp[:, :, sl], start=first, stop=last,
                                perf_mode=DR, skip_group_check=True,
                            )
                            nc.tensor.matmul(
                                ps_d[:, sl], ones_c, pp[:, :, sl],
                                start=first, stop=last,
                                perf_mode=DR, skip_group_check=True,
                            )

                # software-pipelined emission: setup(g+1) ahead of pairs(g);
                # z ops for group 0's pairs go ahead of group 1's copies
                emit_setup(0)
                for _te in range(4):
                    zpe = zz_pool.tile([128, 2, ROWS], BF16, tag=f"zpe{_te}", bufs=1, name=f"zpe{_te}")
                    z_early[_te] = zpe
                    emit_z(_te, zpe)
                for g in range(NG):
                    if g + 1 < NG:
                        emit_setup(g + 1)
                    for t in range(4 * g, 4 * g + 4):
                        emit_pair(t)

            # ---------------- epilogue ----------------
            with tc.tile_pool(name="psE", bufs=2, space="PSUM") as psE:
                den_row = epi.tile([1, ROWS], FP32, tag="den")
                nc.scalar.copy(den_row, ps_d[0:1, :])
                ps_dc = psE.tile([128, 8], FP32, tag="dc")
                for q in range(8):
                    nc.tensor.matmul(
                        ps_dc[:, q : q + 1],
                        den_row[:, 128 * q : 128 * (q + 1)], ones1,
                        start=True, stop=True, skip_group_check=True,
                    )
                rec_cols = epi.tile([128, 8], FP32, tag="rec")
                nc.vector.reciprocal(rec_cols, ps_dc)
                num_sb = epi.tile([128, ROWS], FP32, tag="num")
                for q in range(8):
                    qs = slice(128 * q, 128 * (q + 1))
                    nc.vector.tensor_copy(num_sb[:, qs], ps_oT[:, qs])
                of_all = epi.tile([128, 8, F_OUT], FP32, tag="of")
                for q in range(8):
                    ps_f = psE.tile([128, 128], FP32, tag="f")
                    nc.tensor.transpose(
                        ps_f, num_sb[:, 128 * q : 128 * (q + 1)], idn
                    )
                    m0 = epi.tile([128, 128], FP32, tag=f"m0_{q % 2}")
                    nc.vector.tensor_scalar(
                        out=m0, in0=ps_f, scalar1=rec_cols[:, q : q + 1],
                        scalar2=0.0, op0=Alu.mult, op1=Alu.min,
                    )
                    r0 = epi.tile([128, 128], FP32, tag=f"r0_{q % 2}")
                    nc.scalar.activation(
                        out=r0, in_=ps_f, func=Act.Relu,
                        scale=rec_cols[:, q : q + 1],
                    )
                    e0 = epi.tile([128, 128], FP32, tag=f"e0_{q % 2}")
                    nc.scalar.activation(out=e0, in_=m0, func=Act.Exp)
                    nc.vector.scalar_tensor_tensor(
                        out=of_all[:, q, :], in0=e0, scalar=-1.0, in1=r0,
                        op0=Alu.add, op1=Alu.add,
                    )
                    if q == 3:
                        nc.sync.dma_start(
                            out=out_d[0 : 4 * 128, :].rearrange(
                                "(q p) f -> p q f", p=128
                            ),
                            in_=of_all[:, 0:4, :],
                        )
                nc.sync.dma_start(
                    out=out_d[4 * 128 : 8 * 128, :].rearrange(
                        "(q p) f -> p q f", p=128
                    ),
                    in_=of_all[:, 4:8, :],
                )


def kernel(X, A, weight, a, _trace=False, _tmpdir=None):
    X = np.ascontiguousarray(np.asarray(X, dtype=np.float32))
    A = np.ascontiguousarray(np.asarray(A, dtype=np.int32))
    weight = np.ascontiguousarray(np.asarray(weight, dtype=np.float32))
    a = np.ascontiguousarray(np.asarray(a, dtype=np.float32))

    if "nc" not in _cache:
        _cache["nc"] = _build()
    nc = _cache["nc"]

    bf16 = ml_dtypes.bfloat16
    fp8 = ml_dtypes.float8_e4m3

    Xbf = X.astype(bf16)
    # XTg[g, k, p, c] = X[1024 g + c, 128 k + p]
    XTg_base = np.ascontiguousarray(
        Xbf.reshape(NG, ROWS, KC, 128).transpose(0, 2, 3, 1)
    )
    w_bf = weight.astype(bf16)
    ident = np.eye(128, dtype=np.float32)

    in_maps = []
    for c in range(N_CORES):
        i0 = c * ROWS
        # rotate groups so group 0 is this core's own rows; AT8 rows
        # follow the same j-permutation
        perm = [(c + g) % NG for g in range(NG)]
        XTg = np.ascontiguousarray(XTg_base[perm])
        Asl = (A[i0 : i0 + ROWS].astype(np.float32).T * SCALE).astype(fp8)
        AT8 = np.ascontiguousarray(
            Asl.reshape(NG, ROWS, ROWS)[perm].reshape(N, ROWS)
        )
        in_maps.append(
            {
                "XTg": XTg,
                "AT8": AT8,
                "w_in": w_bf,
                "a_vec": a,
                "ident": ident,
            }
        )

    res = run_bass_kernel_spmd(
        nc, in_maps, core_ids=list(range(N_CORES)), trace=_trace, tmpdir=_tmpdir
    )
    out = np.concatenate([res.results[c]["out"] for c in range(N_CORES)], axis=0)
    if _trace:
        kernel._last_results = res
    return out


# revision 90
# speedup vs baseline: 1.0062x; 1.0062x over previous
"""GAT layer (nn_GATLayer) on 8 Trainium2 NeuronCores.

Math (reference):
    Wh = X @ weight                      [N, F]
    s  = Wh @ a[:F];  t = Wh @ a[F:]     [N, 1]
    e  = relu(s_i + t_j)                 [N, N]
    att = softmax(where(A > 0, e, -9e15), axis=1)
    out = elu(att @ Wh)

Kernel formulation (shift-free softmax, exact up to fp rounding):
    p_ij  = A_ij * max(exp(s_i) * exp(t_j), 1)   (exp(relu(x)) = max(exp(x), 1))
    out_i = elu((p_i: @ Wh) / sum_j p_ij)
A global scale c (=1/4) keeps all fp8 operands in e4m3 normal range:
the host mask carries {0, c}, z' = exp(s - ln(1/c)) * exp(t), and every
psum contribution is uniformly c^2-scaled, which cancels in num/den.

Sharding: 1D row partition across 8 cores (1024 rows each). Host-side
prep is layout/dtype only: X^T (bf16, grouped, rotated so group 0 is
own rows), A^T slab per core (fp8 {0, c}, same group rotation), weight
bf16. All model math (Wh, s, t, exp, softmax, aggregation, ELU) runs
on device.

Per-core dataflow, all in [j (partition), i (free)] orientation:
  - setup: w_all[k] = [weight_k | w*a2 | w*a1]; per j-tile pair one PE
    pass gives [Wh | t] (fp8 Wh); s from group-0 X^T; es/s broadcast
    rows via K=1 PE outer products; t transposed to a row for exp.
  - main loop over 32 j-tile pairs, three z sources balanced across
    engines (PE rank-1 outer product into bf16 psum / DVE 4x ptr-mult
    / ACT exp with bias ptr); two mask paths:
      D-pairs: p8 = (c max z) * mask in one DVE op -> 2 DoubleRow
        matmuls (numerator [128 f, 1024 i], denominator [1, 1024]);
      P-pairs: mask-term DR matmuls consume the raw fp8 mask, the
        relu-term r8 = Pool tensor_tensor mult of relu(z - c).
  - epilogue: den -> columns via K=1 matmuls, reciprocal, PE
    transposes to natural [i, f], fused scale+ELU, one output DMA.
"""

import numpy as np
import ml_dtypes

import concourse.bass as bass
import concourse.bacc as bacc
import concourse.mybir as mybir
import concourse.tile as tile
from concourse.bass_utils import run_bass_kernel_spmd

N = 8192
F_IN = 512
F_OUT = 128
N_CORES = 8
ROWS = N // N_CORES          # 1024 rows per core
NT = N // 128                # 64 j tiles
NP = NT // 2                 # 32 j tile pairs
KC = F_IN // 128             # 4 f_in chunks
NG = 8                       # XT groups (8 j-tiles each)

SCALE = 0.25                 # global fp8 range scale c
LNS = float(np.log(1.0 / SCALE))

FP32 = mybir.dt.float32
BF16 = mybir.dt.bfloat16
FP8 = mybir.dt.float8e4
Alu = mybir.AluOpType
Act = mybir.ActivationFunctionType
DR = mybir.MatmulPerfMode.DoubleRow

# --- engine lane tables (tuned against the TimelineSim cost model) ---
# P_PAIRS: pairs routed through the relu decomposition (Pool mask-mult)
P_PAIRS = frozenset({1, 3, 5, 6, 9, 11, 13, 14, 17, 19, 21, 22, 25, 26})
# z source per pair: 'pe' (rank-1 matmul into psum, D-pairs only),
# 'act' (exp), 'dve' (ptr-mult)
_ZSRC = {}
for _t in range(NP):
    _ZSRC[_t] = "dve" if _t in P_PAIRS else "act"
# relu engine for P-pairs: DVE 4x for most, ACT for some
_RELU_ACT = frozenset({3, 11, 19, 27})

_cache = {}


def _build():
    nc = bacc.Bacc("TRN2", target_bir_lowering=False, debug=False,
                   num_devices=N_CORES)

    XTg = nc.dram_tensor("XTg", [NG, KC, 128, ROWS], BF16, kind="ExternalInput")
    AT8 = nc.dram_tensor("AT8", [N, ROWS], FP8, kind="ExternalInput")
    w_in = nc.dram_tensor("w_in", [F_IN, F_OUT], BF16, kind="ExternalInput")
    a_vec = nc.dram_tensor("a_vec", [2 * F_OUT, 1], FP32, kind="ExternalInput")
    ident = nc.dram_tensor("ident", [128, 128], FP32, kind="ExternalInput")
    out_d = nc.dram_tensor("out", [ROWS, F_OUT], FP32, kind="ExternalOutput")

    with tile.TileContext(nc) as tc:
        _body(nc, tc, XTg, AT8, w_in, a_vec, ident, out_d)

    nc.compile()
    return nc


def _body(nc, tc, XTg, AT8, w_in, a_vec, ident, out_d):
    with (
        tc.tile_pool(name="setup", bufs=1) as setup,
        tc.tile_pool(name="xtg", bufs=2) as xtg_pool,
        tc.tile_pool(name="at", bufs=1) as at_pool,
        tc.tile_pool(name="zz", bufs=5) as zz_pool,
        tc.tile_pool(name="pp", bufs=6) as pp_pool,
        tc.tile_pool(name="epi", bufs=1) as epi,
    ):
        setup_psum = tc.tile_pool(name="psA", bufs=1, space="PSUM")
        psA = setup_psum.__enter__()
        # ---------------- setup: weights ----------------
        w_sb = setup.tile([128, KC, 128], BF16)
        nc.sync.dma_start(
            out=w_sb, in_=w_in.rearrange("(k p) f -> p k f", p=128)
        )
        idn = setup.tile([128, 128], FP32)
        nc.sync.dma_start(out=idn, in_=ident[:, :])
        a_cat = setup.tile([128, 2], BF16)
        nc.gpsimd.dma_start(
            out=a_cat, in_=a_vec.rearrange("(h p) o -> p (h o)", p=128)
        )

        # first XT group doubles as own-rows X^T (host rotates groups)
        xtg0 = xtg_pool.tile([128, KC, ROWS], BF16, tag="g0")
        nc.sync.dma_start(out=xtg0, in_=XTg[0].rearrange("k p i -> p k i"))
        idn_bf = setup.tile([128, 128], BF16)
        nc.vector.tensor_copy(idn_bf, idn)

        # w_all[k] = [weight_k | w_t_k | w_s_k]  [128, 130]
        w_all = []
        for k in range(KC):
            wa = setup.tile([128, F_OUT + 2], BF16, tag=f"w_all{k}")
            nc.vector.tensor_copy(wa[:, 0:F_OUT], w_sb[:, k, :])
            ps_wT = psA.tile([128, 128], BF16, tag="wT")
            nc.tensor.transpose(ps_wT, w_sb[:, k, :], idn_bf)
            wT = setup.tile([128, 128], BF16, tag=f"wT{k}")
            nc.vector.tensor_copy(wT, ps_wT)
            ps_a = psA.tile([128, 2], FP32, tag="pa")
            nc.tensor.matmul(ps_a, wT, a_cat, start=True, stop=True)
            # col F_OUT = w_t (a[F:]), col F_OUT+1 = w_s (a[:F])
            nc.vector.tensor_copy(wa[:, F_OUT : F_OUT + 1], ps_a[:, 1:2])
            nc.vector.tensor_copy(wa[:, F_OUT + 1 : F_OUT + 2], ps_a[:, 0:1])
            w_all.append(wa)


        # ---------------- s (own rows) + broadcast rows ----------------
        ps_s = psA.tile([1, ROWS], FP32, tag="ps_s")
        for h in range(2):
            sl = slice(512 * h, 512 * (h + 1))
            for k in range(KC):
                nc.tensor.matmul(
                    ps_s[:, sl],
                    w_all[k][:, F_OUT + 1 : F_OUT + 2],
                    xtg0[:, k, sl],
                    start=(k == 0), stop=(k == KC - 1),
                    skip_group_check=True,
                )
        nls1 = setup.tile([1, 1], FP32, tag="nls1")
        nc.vector.memset(nls1, -LNS)
        es_row = setup.tile([1, ROWS], BF16)
        nc.scalar.activation(out=es_row, in_=ps_s, func=Act.Exp, bias=nls1)
        s_row = setup.tile([1, ROWS], BF16)
        nc.scalar.copy(s_row, ps_s)

        ones_r = setup.tile([1, 128], BF16)
        nc.vector.memset(ones_r, 1.0)
        es_bc = setup.tile([128, ROWS], BF16)
        s_bc = setup.tile([128, ROWS], BF16)
        for h in range(2):
            sl = slice(512 * h, 512 * (h + 1))
            ps_b = psA.tile([128, 512], FP32, tag="bc", bufs=2)
            nc.tensor.matmul(ps_b, ones_r, es_row[:, sl], start=True, stop=True)
            nc.vector.tensor_copy(es_bc[:, sl], ps_b)
            ps_b2 = psA.tile([128, 512], FP32, tag="bc", bufs=2)
            nc.tensor.matmul(ps_b2, ones_r, s_row[:, sl], start=True, stop=True)
            nc.vector.tensor_copy(s_bc[:, sl], ps_b2)

        ones_c = setup.tile([128, 2, 128], FP8)
        nc.vector.memset(ones_c, 1.0)
        ones1 = setup.tile([1, 1], FP32, tag="ones1")
        nc.vector.memset(ones1, 1.0)

        # ---------------- Wh | t for all j tiles ----------------
        wh_all = setup.tile([128, NT, F_OUT], FP8)
        et_cols = setup.tile([128, NT], FP32)
        ts_cols = setup.tile([128, NT], FP32)
        nsc_c = setup.tile([128, 1], FP32, tag="nsc_c")
        nc.vector.memset(nsc_c, -SCALE)
        nsc_ln = setup.tile([128, 1], FP32, tag="nsc_ln")
        nc.vector.memset(nsc_ln, -LNS)

        setup_psum.__exit__(None, None, None)

        with (
            tc.tile_pool(name="psO", bufs=1, space="PSUM") as psO,
            tc.tile_pool(name="psD", bufs=1, space="PSUM") as psD,
        ):
            ps_oT = psO.tile([128, ROWS], FP32)
            ps_d = psD.tile([128, ROWS], FP32)

            with tc.tile_pool(name="psS", bufs=4, space="PSUM") as psS:
                at_tiles = []

                def emit_setup(g):
                    if g == 0:
                        xtg = xtg0
                    else:
                        xtg = xtg_pool.tile([128, KC, ROWS], BF16)
                        nc.sync.dma_start(
                            out=xtg, in_=XTg[g].rearrange("k p i -> p k i")
                        )
                    at = at_pool.tile([128, 8, ROWS], FP8, tag=f"at{g}")
                    at_tiles.append(at)
                    nc.sync.dma_start(
                        out=at,
                        in_=AT8[ROWS * g : ROWS * (g + 1), :].rearrange(
                            "(t p) i -> p t i", p=128
                        ),
                    )
                    for q in range(4):
                        jt0 = 8 * g + 2 * q
                        ps_p = psS.tile([128, 2, F_OUT + 2], FP32)
                        for v in range(2):
                            co = 128 * (2 * q + v)
                            for k in range(KC):
                                nc.tensor.matmul(
                                    ps_p[:, v, :],
                                    xtg[:, k, co : co + 128],
                                    w_all[k],
                                    start=(k == 0), stop=(k == KC - 1),
                                    skip_group_check=True,
                                )
                        # copies (gpsimd cannot access PSUM)
                        nc.scalar.copy(
                            wh_all[:, jt0 : jt0 + 2, :], ps_p[:, :, 0:F_OUT]
                        )
                        nc.scalar.activation(
                            out=et_cols[:, jt0 : jt0 + 2],
                            in_=ps_p[:, :, F_OUT : F_OUT + 1],
                            func=Act.Exp,
                        )
                        nc.vector.tensor_scalar(
                            out=ts_cols[:, jt0 : jt0 + 2],
                            in0=ps_p[:, :, F_OUT : F_OUT + 1],
                            scalar1=-LNS, scalar2=None, op0=Alu.add,
                        )

                z_early = {}

                def emit_z(t, zp):
                    # dve-z P-pairs fuse the clamp: (es*et) max c in one 4x op
                    fuse = t in P_PAIRS and _ZSRC[t] == "dve"
                    for v in range(2):
                        jt = 2 * t + v
                        if _ZSRC[t] == "act":
                            nc.scalar.activation(
                                out=zp[:, v, :], in_=s_bc, func=Act.Exp,
                                bias=ts_cols[:, jt : jt + 1],
                            )
                        elif fuse:
                            nc.vector.tensor_scalar(
                                out=zp[:, v, :], in0=es_bc,
                                scalar1=et_cols[:, jt : jt + 1],
                                scalar2=SCALE, op0=Alu.mult, op1=Alu.max,
                            )
                        else:
                            nc.vector.tensor_scalar(
                                out=zp[:, v, :], in0=es_bc,
                                scalar1=et_cols[:, jt : jt + 1],
                                scalar2=None, op0=Alu.mult,
                            )

                def emit_pair(t):
                    at = at_tiles[t // 4]
                    s0 = 2 * (t % 4)
                    atsl = at[:, s0 : s0 + 2, :]
                    first, last = t == 0, t == NP - 1
                    fused = t in P_PAIRS and _ZSRC[t] == "dve"
                    if t in z_early:
                        zp = z_early[t]
                    elif not fused:
                        zp = zz_pool.tile([128, 2, ROWS], BF16)
                        emit_z(t, zp)
                    if t in P_PAIRS:
                        # clamp fused into the z op for dve-z pairs; Pool
                        # applies the mask multiply (carrier {0, c})
                        if _ZSRC[t] == "dve" and t in z_early:
                            rt = z_early[t]
                        elif _ZSRC[t] == "dve":
                            rt = pp_pool.tile([128, 2, ROWS], BF16, tag="rt")
                            for v in range(2):
                                jt = 2 * t + v
                                nc.vector.tensor_scalar(
                                    out=rt[:, v, :], in0=es_bc,
                                    scalar1=et_cols[:, jt : jt + 1],
                                    scalar2=SCALE, op0=Alu.mult, op1=Alu.max,
                                )
                        else:
                            rt = pp_pool.tile([128, 2, ROWS], BF16, tag="rt")
                            nc.vector.tensor_scalar(
                                out=rt, in0=zp, scalar1=SCALE, scalar2=None,
                                op0=Alu.max,
                            )
                        pp = pp_pool.tile([128, 2, ROWS], FP8, tag="p8")
                        nc.gpsimd.tensor_tensor(
                            out=pp, in0=rt, in1=atsl, op=Alu.mult
                        )
                    else:
                        pp = pp_pool.tile([128, 2, ROWS], FP8, tag="p8")
                        nc.vector.scalar_tensor_tensor(
                            out=pp, in0=zp, scalar=SCALE,
                            in1=atsl, op0=Alu.max, op1=Alu.mult,
                        )
                    if True:
                        for h in range(2):
                            sl = slice(512 * h, 512 * (h + 1))
                            nc.tensor.matmul(
                                ps_oT[:, sl],
                                wh_all[:, 2 * t : 2 * t + 2, :],
                                pp[:, :, sl], start=first, stop=last,
                                perf_mode=DR, skip_group_check=True,
                            )
                            nc.tensor.matmul(
                                ps_d[:, sl], ones_c, pp[:, :, sl],
                                start=first, stop=last,
                                perf_mode=DR, skip_group_check=True,
                            )

                # software-pipelined emission: setup(g+1) ahead of pairs(g);
                # z ops for group 0's pairs go ahead of group 1's copies
                emit_setup(0)
                for _te in range(4):
                    zpe = zz_pool.tile([128, 2, ROWS], BF16, tag=f"zpe{_te}", bufs=1, name=f"zpe{_te}")
                    z_early[_te] = zpe
                    emit_z(_te, zpe)
                for g in range(NG):
                    if g + 1 < NG:
                        emit_setup(g + 1)
                    for t in range(4 * g, 4 * g + 4):
                        emit_pair(t)

            # ---------------- epilogue ----------------
            with tc.tile_pool(name="psE", bufs=2, space="PSUM") as psE:
                den_row = epi.tile([1, ROWS], FP32, tag="den")
                nc.scalar.copy(den_row, ps_d[0:1, :])
                ps_dc = psE.tile([128, 8], FP32, tag="dc")
                for q in range(8):
                    nc.tensor.matmul(
                        ps_dc[:, q : q + 1],
                        den_row[:, 128 * q : 128 * (q + 1)], ones1,
                        start=True, stop=True, skip_group_check=True,
                    )
                rec_cols = epi.tile([128, 8], FP32, tag="rec")
                nc.vector.reciprocal(rec_cols, ps_dc)
                num_sb = epi.tile([128, ROWS], FP32, tag="num")
                for q in range(8):
                    qs = slice(128 * q, 128 * (q + 1))
                    nc.vector.tensor_copy(num_sb[:, qs], ps_oT[:, qs])
                of_all = epi.tile([128, 8, F_OUT], FP32, tag="of")
                for q in range(8):
                    ps_f = psE.tile([128, 128], FP32, tag="f")
                    nc.tensor.transpose(
                        ps_f, num_sb[:, 128 * q : 128 * (q + 1)], idn
                    )
                    m0 = epi.tile([128, 128], FP32, tag=f"m0_{q % 2}")
                    nc.vector.tensor_scalar(
                        out=m0, in0=ps_f, scalar1=rec_cols[:, q : q + 1],
                        scalar2=0.0, op0=Alu.mult, op1=Alu.min,
                    )
                    r0 = epi.tile([128, 128], FP32, tag=f"r0_{q % 2}")
                    nc.scalar.activation(
                        out=r0, in_=ps_f, func=Act.Relu,
                        scale=rec_cols[:, q : q + 1],
                    )
                    e0 = epi.tile([128, 128], FP32, tag=f"e0_{q % 2}")
                    nc.scalar.activation(out=e0, in_=m0, func=Act.Exp)
                    nc.vector.scalar_tensor_tensor(
                        out=of_all[:, q, :], in0=e0, scalar=-1.0, in1=r0,
                        op0=Alu.add, op1=Alu.add,
                    )
                    if q == 3:
                        nc.sync.dma_start(
                            out=out_d[0 : 4 * 128, :].rearrange(
                                "(q p) f -> p q f", p=128
                            ),
                            in_=of_all[:, 0:4, :],
                        )
                nc.sync.dma_start(
                    out=out_d[4 * 128 : 8 * 128, :].rearrange(
                        "(q p) f -> p q f", p=128
                    ),
                    in_=of_all[:, 4:8, :],
                )


def kernel(X, A, weight, a, _trace=False, _tmpdir=None):
    X = np.ascontiguousarray(np.asarray(X, dtype=np.float32))
    A = np.ascontiguousarray(np.asarray(A, dtype=np.int32))
    weight = np.ascontiguousarray(np.asarray(weight, dtype=np.float32))
    a = np.ascontiguousarray(np.asarray(a, dtype=np.float32))

    if "nc" not in _cache:
        _cache["nc"] = _build()
    nc = _cache["nc"]

    bf16 = ml_dtypes.bfloat16
    fp8 = ml_dtypes.float8_e4m3

    Xbf = X.astype(bf16)
    # XTg[g, k, p, c] = X[1024 g + c, 128 k + p]
    XTg_base = np.ascontiguousarray(
        Xbf.reshape(NG, ROWS, KC, 128).transpose(0, 2, 3, 1)
    )
    w_bf = weight.astype(bf16)
    ident = np.eye(128, dtype=np.float32)

    in_maps = []
    for c in range(N_CORES):
        i0 = c * ROWS
        # rotate groups so group 0 is this core's own rows; AT8 rows
        # follow the same j-permutation
        perm = [(c + g) % NG for g in range(NG)]
        XTg = np.ascontiguousarray(XTg_base[perm])
        Asl = (A[i0 : i0 + ROWS].astype(np.float32).T * SCALE).astype(fp8)
        AT8 = np.ascontiguousarray(
            Asl.reshape(NG, ROWS, ROWS)[perm].reshape(N, ROWS)
        )
        in_maps.append(
            {
                "XTg": XTg,
                "AT8": AT8,
                "w_in": w_bf,
                "a_vec": a,
                "ident": ident,
            }
        )

    res = run_bass_kernel_spmd(
        nc, in_maps, core_ids=list(range(N_CORES)), trace=_trace, tmpdir=_tmpdir
    )
    out = np.concatenate([res.results[c]["out"] for c in range(N_CORES)], axis=0)
    if _trace:
        kernel._last_results = res
    return out


# revision 105
# speedup vs baseline: 1.0201x; 1.0138x over previous
"""GAT layer (nn_GATLayer) on 8 Trainium2 NeuronCores.

Math (reference):
    Wh = X @ weight                      [N, F]
    s  = Wh @ a[:F];  t = Wh @ a[F:]     [N, 1]
    e  = relu(s_i + t_j)                 [N, N]
    att = softmax(where(A > 0, e, -9e15), axis=1)
    out = elu(att @ Wh)

Kernel formulation (shift-free softmax, exact up to fp rounding):
    p_ij  = A_ij * max(exp(s_i) * exp(t_j), 1)   (exp(relu(x)) = max(exp(x), 1))
    out_i = elu((p_i: @ Wh) / sum_j p_ij)
A global scale c (=1/4) keeps all fp8 operands in e4m3 normal range:
the host mask carries {0, c}, z' = exp(s - ln(1/c)) * exp(t), and every
psum contribution is uniformly c^2-scaled, which cancels in num/den.

Sharding: 1D row partition across 8 cores (1024 rows each). Host-side
prep is layout/dtype only: X^T (bf16, grouped, rotated so group 0 is
own rows), A^T slab per core (fp8 {0, c}, same group rotation), weight
bf16. All model math (Wh, s, t, exp, softmax, aggregation, ELU) runs
on device.

Per-core dataflow, all in [j (partition), i (free)] orientation:
  - setup: w_all[k] = [weight_k | w*a2 | w*a1]; per j-tile pair one PE
    pass gives [Wh | t] (fp8 Wh); s from group-0 X^T; es/s broadcast
    rows via K=1 PE outer products; t transposed to a row for exp.
  - main loop over 32 j-tile pairs, three z sources balanced across
    engines (PE rank-1 outer product into bf16 psum / DVE 4x ptr-mult
    / ACT exp with bias ptr); two mask paths:
      D-pairs: p8 = (c max z) * mask in one DVE op -> 2 DoubleRow
        matmuls (numerator [128 f, 1024 i], denominator [1, 1024]);
      P-pairs: mask-term DR matmuls consume the raw fp8 mask, the
        relu-term r8 = Pool tensor_tensor mult of relu(z - c).
  - epilogue: den -> columns via K=1 matmuls, reciprocal, PE
    transposes to natural [i, f], fused scale+ELU, one output DMA.
"""

import numpy as np
import ml_dtypes

import concourse.bass as bass
import concourse.bacc as bacc
import concourse.mybir as mybir
import concourse.tile as tile
from concourse.bass_utils import run_bass_kernel_spmd

N = 8192
F_IN = 512
F_OUT = 128
N_CORES = 8
ROWS = N // N_CORES          # 1024 rows per core
NT = N // 128                # 64 j tiles
NP = NT // 2                 # 32 j tile pairs
KC = F_IN // 128             # 4 f_in chunks
NG = 8                       # XT groups (8 j-tiles each)

SCALE = 0.25                 # global fp8 range scale c
LNS = float(np.log(1.0 / SCALE))

FP32 = mybir.dt.float32
BF16 = mybir.dt.bfloat16
FP8 = mybir.dt.float8e4
Alu = mybir.AluOpType
Act = mybir.ActivationFunctionType
DR = mybir.MatmulPerfMode.DoubleRow

# --- engine lane tables (tuned against the TimelineSim cost model) ---
# P_PAIRS: pairs routed through the relu decomposition (Pool mask-mult)
P_PAIRS = frozenset({1, 3, 5, 6, 9, 11, 13, 14, 17, 19, 21, 22, 25, 26})
# z source per pair: 'pe' (rank-1 matmul into psum, D-pairs only),
# 'act' (exp), 'dve' (ptr-mult)
_ZSRC = {}
for _t in range(NP):
    _ZSRC[_t] = "dve" if _t in P_PAIRS else "act"
# relu engine for P-pairs: DVE 4x for most, ACT for some
_RELU_ACT = frozenset({3, 11, 19, 27})

_cache = {}


def _build():
    nc = bacc.Bacc("TRN2", target_bir_lowering=False, debug=False,
                   num_devices=N_CORES)

    XTg = nc.dram_tensor("XTg", [NG, KC, 128, ROWS], BF16, kind="ExternalInput")
    AT8 = nc.dram_tensor("AT8", [N, ROWS], FP8, kind="ExternalInput")
    w_in = nc.dram_tensor("w_in", [F_IN, F_OUT], BF16, kind="ExternalInput")
    a_vec = nc.dram_tensor("a_vec", [2 * F_OUT, 1], FP32, kind="ExternalInput")
    ident = nc.dram_tensor("ident", [128, 128], FP32, kind="ExternalInput")
    out_d = nc.dram_tensor("out", [ROWS, F_OUT], FP32, kind="ExternalOutput")

    with tile.TileContext(nc) as tc:
        _body(nc, tc, XTg, AT8, w_in, a_vec, ident, out_d)

    nc.compile()
    return nc


def _body(nc, tc, XTg, AT8, w_in, a_vec, ident, out_d):
    with (
        tc.tile_pool(name="setup", bufs=1) as setup,
        tc.tile_pool(name="xtg", bufs=2) as xtg_pool,
        tc.tile_pool(name="at", bufs=1) as at_pool,
        tc.tile_pool(name="zz", bufs=5) as zz_pool,
        tc.tile_pool(name="pp", bufs=6) as pp_pool,
        tc.tile_pool(name="epi", bufs=1) as epi,
    ):
        setup_psum = tc.tile_pool(name="psA", bufs=1, space="PSUM")
        psA = setup_psum.__enter__()
        # ---------------- setup: weights ----------------
        w_sb = setup.tile([128, KC, 128], BF16)
        nc.sync.dma_start(
            out=w_sb, in_=w_in.rearrange("(k p) f -> p k f", p=128)
        )
        idn = setup.tile([128, 128], FP32)
        nc.sync.dma_start(out=idn, in_=ident[:, :])
        a_cat = setup.tile([128, 2], BF16)
        nc.gpsimd.dma_start(
            out=a_cat, in_=a_vec.rearrange("(h p) o -> p (h o)", p=128)
        )

        # first XT group doubles as own-rows X^T (host rotates groups)
        xtg0 = xtg_pool.tile([128, KC, ROWS], BF16, tag="g0")
        for k in range(KC):
            nc.sync.dma_start(
                out=xtg0[:, k, :],
                in_=XTg[0].rearrange("k p i -> p k i")[:, k, :],
            )
        idn_bf = setup.tile([128, 128], BF16)
        nc.vector.tensor_copy(idn_bf, idn)

        # w_all[k] = [weight_k | w_t_k | w_s_k]  [128, 130]
        w_all = []
        for k in range(KC):
            wa = setup.tile([128, F_OUT + 2], BF16, tag=f"w_all{k}")
            nc.vector.tensor_copy(wa[:, 0:F_OUT], w_sb[:, k, :])
            ps_wT = psA.tile([128, 128], BF16, tag="wT")
            nc.tensor.transpose(ps_wT, w_sb[:, k, :], idn_bf)
            wT = setup.tile([128, 128], BF16, tag=f"wT{k}")
            nc.vector.tensor_copy(wT, ps_wT)
            ps_a = psA.tile([128, 2], FP32, tag="pa")
            nc.tensor.matmul(ps_a, wT, a_cat, start=True, stop=True)
            # col F_OUT = w_t (a[F:]), col F_OUT+1 = w_s (a[:F])
            nc.vector.tensor_copy(wa[:, F_OUT : F_OUT + 1], ps_a[:, 1:2])
            nc.vector.tensor_copy(wa[:, F_OUT + 1 : F_OUT + 2], ps_a[:, 0:1])
            w_all.append(wa)


        # ---------------- s (own rows) + broadcast rows ----------------
        ps_s = psA.tile([1, ROWS], FP32, tag="ps_s")
        for h in range(2):
            sl = slice(512 * h, 512 * (h + 1))
            for k in range(KC):
                nc.tensor.matmul(
                    ps_s[:, sl],
                    w_all[k][:, F_OUT + 1 : F_OUT + 2],
                    xtg0[:, k, sl],
                    start=(k == 0), stop=(k == KC - 1),
                    skip_group_check=True,
                )
        nls1 = setup.tile([1, 1], FP32, tag="nls1")
        nc.vector.memset(nls1, -LNS)
        es_row = setup.tile([1, ROWS], BF16)
        nc.scalar.activation(out=es_row, in_=ps_s, func=Act.Exp, bias=nls1)
        s_row = setup.tile([1, ROWS], BF16)
        nc.vector.tensor_copy(s_row, ps_s)

        ones_r = setup.tile([1, 128], BF16)
        nc.vector.memset(ones_r, 1.0)
        es_bc = setup.tile([128, ROWS], BF16)
        s_bc = setup.tile([128, ROWS], BF16)
        for h in range(2):
            sl = slice(512 * h, 512 * (h + 1))
            ps_b = psA.tile([128, 512], FP32, tag="bc", bufs=2)
            nc.tensor.matmul(ps_b, ones_r, es_row[:, sl], start=True, stop=True)
            nc.vector.tensor_copy(es_bc[:, sl], ps_b)
            ps_b2 = psA.tile([128, 512], FP32, tag="bc", bufs=2)
            nc.tensor.matmul(ps_b2, ones_r, s_row[:, sl], start=True, stop=True)
            nc.scalar.copy(s_bc[:, sl], ps_b2)

        ones_c = setup.tile([128, 2, 128], FP8)
        nc.vector.memset(ones_c, 1.0)
        ones1 = setup.tile([1, 1], FP32, tag="ones1")
        nc.vector.memset(ones1, 1.0)

        # ---------------- Wh | t for all j tiles ----------------
        wh_all = setup.tile([128, NT, F_OUT], FP8)
        et_cols = setup.tile([128, NT], FP32)
        ts_cols = setup.tile([128, NT], FP32)
        nsc_c = setup.tile([128, 1], FP32, tag="nsc_c")
        nc.vector.memset(nsc_c, -SCALE)
        nsc_ln = setup.tile([128, 1], FP32, tag="nsc_ln")
        nc.vector.memset(nsc_ln, -LNS)

        setup_psum.__exit__(None, None, None)

        with (
            tc.tile_pool(name="psO", bufs=1, space="PSUM") as psO,
            tc.tile_pool(name="psD", bufs=1, space="PSUM") as psD,
        ):
            ps_oT = psO.tile([128, ROWS], FP32)
            ps_d = psD.tile([128, ROWS], FP32)

            with tc.tile_pool(name="psS", bufs=4, space="PSUM") as psS:
                at_tiles = []

                def emit_setup(g):
                    if g == 0:
                        xtg = xtg0
                    else:
                        xtg = xtg_pool.tile([128, KC, ROWS], BF16)
                        nc.sync.dma_start(
                            out=xtg, in_=XTg[g].rearrange("k p i -> p k i")
                        )
                    at = at_pool.tile([128, 8, ROWS], FP8, tag=f"at{g}")
                    at_tiles.append(at)
                    nc.sync.dma_start(
                        out=at,
                        in_=AT8[ROWS * g : ROWS * (g + 1), :].rearrange(
                            "(t p) i -> p t i", p=128
                        ),
                    )
                    for q in range(4):
                        jt0 = 8 * g + 2 * q
                        ps_p = psS.tile([128, 2, F_OUT + 2], FP32)
                        for v in range(2):
                            co = 128 * (2 * q + v)
                            for k in range(KC):
                                nc.tensor.matmul(
                                    ps_p[:, v, :],
                                    xtg[:, k, co : co + 128],
                                    w_all[k],
                                    start=(k == 0), stop=(k == KC - 1),
                                    skip_group_check=True,
                                )
                        # copies (gpsimd cannot access PSUM)
                        nc.scalar.copy(
                            wh_all[:, jt0 : jt0 + 2, :], ps_p[:, :, 0:F_OUT]
                        )
                        nc.scalar.activation(
                            out=et_cols[:, jt0 : jt0 + 2],
                            in_=ps_p[:, :, F_OUT : F_OUT + 1],
                            func=Act.Exp,
                        )
                        nc.vector.tensor_scalar(
                            out=ts_cols[:, jt0 : jt0 + 2],
                            in0=ps_p[:, :, F_OUT : F_OUT + 1],
                            scalar1=-LNS, scalar2=None, op0=Alu.add,
                        )

                z_early = {}

                def emit_z(t, zp):
                    # dve-z P-pairs fuse the clamp: (es*et) max c in one 4x op
                    fuse = t in P_PAIRS and _ZSRC[t] == "dve"
                    for v in range(2):
                        jt = 2 * t + v
                        if _ZSRC[t] == "act":
                            nc.scalar.activation(
                                out=zp[:, v, :], in_=s_bc, func=Act.Exp,
                                bias=ts_cols[:, jt : jt + 1],
                            )
                        elif fuse:
                            nc.vector.tensor_scalar(
                                out=zp[:, v, :], in0=es_bc,
                                scalar1=et_cols[:, jt : jt + 1],
                                scalar2=SCALE, op0=Alu.mult, op1=Alu.max,
                            )
                        else:
                            nc.vector.tensor_scalar(
                                out=zp[:, v, :], in0=es_bc,
                                scalar1=et_cols[:, jt : jt + 1],
                                scalar2=None, op0=Alu.mult,
                            )

                def emit_pair(t):
                    at = at_tiles[t // 4]
                    s0 = 2 * (t % 4)
                    atsl = at[:, s0 : s0 + 2, :]
                    first, last = t == 0, t == NP - 1
                    fused = t in P_PAIRS and _ZSRC[t] == "dve"
                    if t in z_early:
                        zp = z_early[t]
                    elif not fused:
                        zp = zz_pool.tile([128, 2, ROWS], BF16)
                        emit_z(t, zp)
                    if t in P_PAIRS:
                        # clamp fused into the z op for dve-z pairs; Pool
                        # applies the mask multiply (carrier {0, c})
                        if _ZSRC[t] == "dve" and t in z_early:
                            rt = z_early[t]
                        elif _ZSRC[t] == "dve":
                            rt = pp_pool.tile([128, 2, ROWS], BF16, tag="rt")
                            for v in range(2):
                                jt = 2 * t + v
                                nc.vector.tensor_scalar(
                                    out=rt[:, v, :], in0=es_bc,
                                    scalar1=et_cols[:, jt : jt + 1],
                                    scalar2=SCALE, op0=Alu.mult, op1=Alu.max,
                                )
                        else:
                            rt = pp_pool.tile([128, 2, ROWS], BF16, tag="rt")
                            nc.vector.tensor_scalar(
                                out=rt, in0=zp, scalar1=SCALE, scalar2=None,
                                op0=Alu.max,
                            )
                        pp = pp_pool.tile([128, 2, ROWS], FP8, tag="p8")
                        nc.gpsimd.tensor_tensor(
                            out=pp, in0=rt, in1=atsl, op=Alu.mult
                        )
                    else:
                        pp = pp_pool.tile([128, 2, ROWS], FP8, tag="p8")
                        nc.vector.scalar_tensor_tensor(
                            out=pp, in0=zp, scalar=SCALE,
                            in1=atsl, op0=Alu.max, op1=Alu.mult,
                        )
                    if True:
                        for h in range(2):
                            sl = slice(512 * h, 512 * (h + 1))
                            nc.tensor.matmul(
                                ps_oT[:, sl],
                                wh_all[:, 2 * t : 2 * t + 2, :],
                                pp[:, :, sl], start=first, stop=last,
                                perf_mode=DR, skip_group_check=True,
                            )
                            nc.tensor.matmul(
                                ps_d[:, sl], ones_c, pp[:, :, sl],
                                start=first, stop=last,
                                perf_mode=DR, skip_group_check=True,
                            )

                # software-pipelined emission: setup(g+1) ahead of pairs(g);
                # z ops for group 0's pairs go ahead of group 1's copies
                emit_setup(0)
                for _te in range(4):
                    zpe = zz_pool.tile([128, 2, ROWS], BF16, tag=f"zpe{_te}", bufs=1, name=f"zpe{_te}")
                    z_early[_te] = zpe
                    emit_z(_te, zpe)
                for g in range(NG):
                    if g + 1 < NG:
                        emit_setup(g + 1)
                    for t in range(4 * g, 4 * g + 4):
                        emit_pair(t)

            # ---------------- epilogue ----------------
            with tc.tile_pool(name="psE", bufs=2, space="PSUM") as psE:
                den_row = epi.tile([1, ROWS], FP32, tag="den")
                nc.scalar.copy(den_row, ps_d[0:1, :])
                ps_dc = psE.tile([128, 8], FP32, tag="dc")
                for q in range(8):
                    nc.tensor.matmul(
                        ps_dc[:, q : q + 1],
                        den_row[:, 128 * q : 128 * (q + 1)], ones1,
                        start=True, stop=True, skip_group_check=True,
                    )
                rec_cols = epi.tile([128, 8], FP32, tag="rec")
                nc.vector.reciprocal(rec_cols, ps_dc)
                num_sb = epi.tile([128, ROWS], FP32, tag="num")
                for q in range(8):
                    qs = slice(128 * q, 128 * (q + 1))
                    nc.vector.tensor_copy(num_sb[:, qs], ps_oT[:, qs])
                of_all = epi.tile([128, 8, F_OUT], FP32, tag="of")
                for q in range(8):
                    ps_f = psE.tile([128, 128], FP32, tag="f")
                    nc.tensor.transpose(
                        ps_f, num_sb[:, 128 * q : 128 * (q + 1)], idn
                    )
                    m0 = epi.tile([128, 128], FP32, tag=f"m0_{q % 2}")
                    nc.vector.tensor_scalar(
                        out=m0, in0=ps_f, scalar1=rec_cols[:, q : q + 1],
                        scalar2=0.0, op0=Alu.mult, op1=Alu.min,
                    )
                    r0 = epi.tile([128, 128], FP32, tag=f"r0_{q % 2}")
                    nc.scalar.activation(
                        out=r0, in_=ps_f, func=Act.Relu,
                        scale=rec_cols[:, q : q + 1],
                    )
                    e0 = epi.tile([128, 128], FP32, tag=f"e0_{q % 2}")
                    nc.scalar.activation(out=e0, in_=m0, func=Act.Exp)
                    nc.vector.scalar_tensor_tensor(
                        out=of_all[:, q, :], in0=e0, scalar=-1.0, in1=r0,
                        op0=Alu.add, op1=Alu.add,
                    )
                    if q == 3:
                        nc.sync.dma_start(
                            out=out_d[0 : 4 * 128, :].rearrange(
                                "(q p) f -> p q f", p=128
                            ),
                            in_=of_all[:, 0:4, :],
                        )
                nc.sync.dma_start(
                    out=out_d[4 * 128 : 8 * 128, :].rearrange(
                        "(q p) f -> p q f", p=128
                    ),
                    in_=of_all[:, 4:8, :],
                )


def kernel(X, A, weight, a, _trace=False, _tmpdir=None):
    X = np.ascontiguousarray(np.asarray(X, dtype=np.float32))
    A = np.ascontiguousarray(np.asarray(A, dtype=np.int32))
    weight = np.ascontiguousarray(np.asarray(weight, dtype=np.float32))
    a = np.ascontiguousarray(np.asarray(a, dtype=np.float32))

    if "nc" not in _cache:
        _cache["nc"] = _build()
    nc = _cache["nc"]

    bf16 = ml_dtypes.bfloat16
    fp8 = ml_dtypes.float8_e4m3

    Xbf = X.astype(bf16)
    # XTg[g, k, p, c] = X[1024 g + c, 128 k + p]
    XTg_base = np.ascontiguousarray(
        Xbf.reshape(NG, ROWS, KC, 128).transpose(0, 2, 3, 1)
    )
    w_bf = weight.astype(bf16)
    ident = np.eye(128, dtype=np.float32)

    in_maps = []
    for c in range(N_CORES):
        i0 = c * ROWS
        # rotate groups so group 0 is this core's own rows; AT8 rows
        # follow the same j-permutation
        perm = [(c + g) % NG for g in range(NG)]
        XTg = np.ascontiguousarray(XTg_base[perm])
        Asl = (A[i0 : i0 + ROWS].astype(np.float32).T * SCALE).astype(fp8)
        AT8 = np.ascontiguousarray(
            Asl.reshape(NG, ROWS, ROWS)[perm].reshape(N, ROWS)
        )
        in_maps.append(
            {
                "XTg": XTg,
                "AT8": AT8,
                "w_in": w_bf,
                "a_vec": a,
                "ident": ident,
            }
        )

    res = run_bass_kernel_spmd(
        nc, in_maps, core_ids=list(range(N_CORES)), trace=_trace, tmpdir=_tmpdir
    )
    out = np.concatenate([res.results[c]["out"] for c in range(N_CORES)], axis=0)
    if _trace:
        kernel._last_results = res
    return out


# revision 109
# speedup vs baseline: 1.0497x; 1.0291x over previous
"""GAT layer (nn_GATLayer) on 8 Trainium2 NeuronCores.

Math (reference):
    Wh = X @ weight                      [N, F]
    s  = Wh @ a[:F];  t = Wh @ a[F:]     [N, 1]
    e  = relu(s_i + t_j)                 [N, N]
    att = softmax(where(A > 0, e, -9e15), axis=1)
    out = elu(att @ Wh)

Kernel formulation (shift-free softmax, exact up to fp rounding):
    p_ij  = A_ij * max(exp(s_i) * exp(t_j), 1)   (exp(relu(x)) = max(exp(x), 1))
    out_i = elu((p_i: @ Wh) / sum_j p_ij)
A global scale c (=1/4) keeps all fp8 operands in e4m3 normal range:
the host mask carries {0, c}, z' = exp(s - ln(1/c)) * exp(t), and every
psum contribution is uniformly c^2-scaled, which cancels in num/den.

Sharding: 1D row partition across 8 cores (1024 rows each). Host-side
prep is layout/dtype only: X^T (bf16, grouped, rotated so group 0 is
own rows), A^T slab per core (fp8 {0, c}, same group rotation), weight
bf16. All model math (Wh, s, t, exp, softmax, aggregation, ELU) runs
on device.

Per-core dataflow, all in [j (partition), i (free)] orientation:
  - setup: w_all[k] = [weight_k | w*a2 | w*a1]; per j-tile pair one PE
    pass gives [Wh | t] (fp8 Wh); s from group-0 X^T; es/s broadcast
    rows via K=1 PE outer products; t transposed to a row for exp.
  - main loop over 32 j-tile pairs, three z sources balanced across
    engines (PE rank-1 outer product into bf16 psum / DVE 4x ptr-mult
    / ACT exp with bias ptr); two mask paths:
      D-pairs: p8 = (c max z) * mask in one DVE op -> 2 DoubleRow
        matmuls (numerator [128 f, 1024 i], denominator [1, 1024]);
      P-pairs: mask-term DR matmuls consume the raw fp8 mask, the
        relu-term r8 = Pool tensor_tensor mult of relu(z - c).
  - epilogue: den -> columns via K=1 matmuls, reciprocal, PE
    transposes to natural [i, f], fused scale+ELU, one output DMA.
"""

import numpy as np
import ml_dtypes

import concourse.bass as bass
import concourse.bacc as bacc
import concourse.mybir as mybir
import concourse.tile as tile
from concourse.bass_utils import run_bass_kernel_spmd

N = 8192
F_IN = 512
F_OUT = 128
N_CORES = 8
ROWS = N // N_CORES          # 1024 rows per core
NT = N // 128                # 64 j tiles
NP = NT // 2                 # 32 j tile pairs
KC = F_IN // 128             # 4 f_in chunks
NG = 8                       # XT groups (8 j-tiles each)

SCALE = 0.25                 # global fp8 range scale c
LNS = float(np.log(1.0 / SCALE))

FP32 = mybir.dt.float32
BF16 = mybir.dt.bfloat16
FP8 = mybir.dt.float8e4
Alu = mybir.AluOpType
Act = mybir.ActivationFunctionType
DR = mybir.MatmulPerfMode.DoubleRow

# --- engine lane tables (tuned against the TimelineSim cost model) ---
# P_PAIRS: pairs routed through the relu decomposition (Pool mask-mult)
P_PAIRS = frozenset({1, 3, 5, 6, 9, 11, 13, 14, 17, 19, 21, 22, 25, 26})
# z source per pair: 'pe' (rank-1 matmul into psum, D-pairs only),
# 'act' (exp), 'dve' (ptr-mult)
_ZSRC = {}
for _t in range(NP):
    _ZSRC[_t] = "dve" if _t in P_PAIRS else "act"
# relu engine for P-pairs: DVE 4x for most, ACT for some
_RELU_ACT = frozenset({3, 11, 19, 27})

_cache = {}


def _build():
    nc = bacc.Bacc("TRN2", target_bir_lowering=False, debug=False,
                   num_devices=N_CORES)

    XTg = nc.dram_tensor("XTg", [NG, KC, 128, ROWS], BF16, kind="ExternalInput")
    AT8 = nc.dram_tensor("AT8", [N, ROWS], FP8, kind="ExternalInput")
    w_in = nc.dram_tensor("w_in", [F_IN, F_OUT], BF16, kind="ExternalInput")
    a_vec = nc.dram_tensor("a_vec", [2 * F_OUT, 1], FP32, kind="ExternalInput")
    ident = nc.dram_tensor("ident", [128, 128], FP32, kind="ExternalInput")
    out_d = nc.dram_tensor("out", [ROWS, F_OUT], FP32, kind="ExternalOutput")

    with tile.TileContext(nc) as tc:
        _body(nc, tc, XTg, AT8, w_in, a_vec, ident, out_d)

    nc.compile()
    return nc


def _body(nc, tc, XTg, AT8, w_in, a_vec, ident, out_d):
    with (
        tc.tile_pool(name="setup", bufs=1) as setup,
        tc.tile_pool(name="xtg", bufs=2) as xtg_pool,
        tc.tile_pool(name="at", bufs=1) as at_pool,
        tc.tile_pool(name="zz", bufs=5) as zz_pool,
        tc.tile_pool(name="pp", bufs=6) as pp_pool,
        tc.tile_pool(name="epi", bufs=1) as epi,
    ):
        setup_psum = tc.tile_pool(name="psA", bufs=1, space="PSUM")
        psA = setup_psum.__enter__()
        psW_cm = tc.tile_pool(name="psW", bufs=1, space="PSUM")
        psW = psW_cm.__enter__()
        # ---------------- setup: weights ----------------
        w_sb = setup.tile([128, KC, 128], BF16)
        nc.sync.dma_start(
            out=w_sb, in_=w_in.rearrange("(k p) f -> p k f", p=128)
        )
        idn = setup.tile([128, 128], FP32)
        nc.sync.dma_start(out=idn, in_=ident[:, :])
        a_cat = setup.tile([128, 2], BF16)
        nc.gpsimd.dma_start(
            out=a_cat, in_=a_vec.rearrange("(h p) o -> p (h o)", p=128)
        )

        # first XT group doubles as own-rows X^T (host rotates groups)
        xtg0 = xtg_pool.tile([128, KC, ROWS], BF16, tag="g0")
        for k in range(KC):
            nc.sync.dma_start(
                out=xtg0[:, k, :],
                in_=XTg[0].rearrange("k p i -> p k i")[:, k, :],
            )
        idn_bf = setup.tile([128, 128], BF16)
        nc.vector.tensor_copy(idn_bf, idn)

        # w_all[k] = [weight_k | w_t_k | w_s_k]  [128, 130]
        w_all = []
        for k in range(KC):
            wa = setup.tile([128, F_OUT + 2], BF16, tag=f"w_all{k}")
            nc.vector.tensor_copy(wa[:, 0:F_OUT], w_sb[:, k, :])
            ps_wT = psW.tile([128, 128], BF16, tag="wT")
            nc.tensor.transpose(ps_wT, w_sb[:, k, :], idn_bf)
            wT = setup.tile([128, 128], BF16, tag=f"wT{k}")
            nc.vector.tensor_copy(wT, ps_wT)
            ps_a = psW.tile([128, 2], FP32, tag="pa")
            nc.tensor.matmul(ps_a, wT, a_cat, start=True, stop=True)
            # col F_OUT = w_t (a[F:]), col F_OUT+1 = w_s (a[:F])
            nc.vector.tensor_copy(wa[:, F_OUT : F_OUT + 1], ps_a[:, 1:2])
            nc.vector.tensor_copy(wa[:, F_OUT + 1 : F_OUT + 2], ps_a[:, 0:1])
            w_all.append(wa)
        psW_cm.__exit__(None, None, None)


        # ---------------- s (own rows) + broadcast rows ----------------
        ps_s = psA.tile([1, ROWS], FP32, tag="ps_s")
        for h in range(2):
            sl = slice(512 * h, 512 * (h + 1))
            for k in range(KC):
                nc.tensor.matmul(
                    ps_s[:, sl],
                    w_all[k][:, F_OUT + 1 : F_OUT + 2],
                    xtg0[:, k, sl],
                    start=(k == 0), stop=(k == KC - 1),
                    skip_group_check=True,
                )
        nls1 = setup.tile([1, 1], FP32, tag="nls1")
        nc.vector.memset(nls1, -LNS)
        es_row = setup.tile([1, ROWS], BF16)
        nc.scalar.activation(out=es_row, in_=ps_s, func=Act.Exp, bias=nls1)
        s_row = setup.tile([1, ROWS], BF16)
        nc.vector.tensor_copy(s_row, ps_s)

        ones_r = setup.tile([1, 128], BF16)
        nc.vector.memset(ones_r, 1.0)
        es_bc = setup.tile([128, ROWS], BF16)
        s_bc = setup.tile([128, ROWS], BF16)
        for h in range(2):
            sl = slice(512 * h, 512 * (h + 1))
            ps_b = psA.tile([128, 512], FP32, tag="bc", bufs=2)
            nc.tensor.matmul(ps_b, ones_r, es_row[:, sl], start=True, stop=True)
            nc.vector.tensor_copy(es_bc[:, sl], ps_b)
            ps_b2 = psA.tile([128, 512], FP32, tag="bc", bufs=2)
            nc.tensor.matmul(ps_b2, ones_r, s_row[:, sl], start=True, stop=True)
            nc.scalar.copy(s_bc[:, sl], ps_b2)

        ones_c = setup.tile([128, 2, 128], FP8)
        nc.vector.memset(ones_c, 1.0)
        ones1 = setup.tile([1, 1], FP32, tag="ones1")
        nc.vector.memset(ones1, 1.0)

        # ---------------- Wh | t for all j tiles ----------------
        wh_all = setup.tile([128, NT, F_OUT], FP8)
        et_cols = setup.tile([128, NT], FP32)
        ts_cols = setup.tile([128, NT], FP32)
        nsc_c = setup.tile([128, 1], FP32, tag="nsc_c")
        nc.vector.memset(nsc_c, -SCALE)
        nsc_ln = setup.tile([128, 1], FP32, tag="nsc_ln")
        nc.vector.memset(nsc_ln, -LNS)

        setup_psum.__exit__(None, None, None)

        with (
            tc.tile_pool(name="psO", bufs=1, space="PSUM") as psO,
            tc.tile_pool(name="psD", bufs=1, space="PSUM") as psD,
        ):
            ps_oT = psO.tile([128, ROWS], FP32)
            ps_d = psD.tile([128, ROWS], FP32)

            with tc.tile_pool(name="psS", bufs=4, space="PSUM") as psS:
                at_tiles = []

                def emit_setup(g):
                    if g == 0:
                        xtg = xtg0
                    else:
                        xtg = xtg_pool.tile([128, KC, ROWS], BF16)
                        nc.sync.dma_start(
                            out=xtg, in_=XTg[g].rearrange("k p i -> p k i")
                        )
                    at = at_pool.tile([128, 8, ROWS], FP8, tag=f"at{g}")
                    at_tiles.append(at)
                    nc.sync.dma_start(
                        out=at,
                        in_=AT8[ROWS * g : ROWS * (g + 1), :].rearrange(
                            "(t p) i -> p t i", p=128
                        ),
                    )
                    for q in range(4):
                        jt0 = 8 * g + 2 * q
                        ps_p = psS.tile([128, 2, F_OUT + 2], FP32)
                        for v in range(2):
                            co = 128 * (2 * q + v)
                            for k in range(KC):
                                nc.tensor.matmul(
                                    ps_p[:, v, :],
                                    xtg[:, k, co : co + 128],
                                    w_all[k],
                                    start=(k == 0), stop=(k == KC - 1),
                                    skip_group_check=True,
                                )
                        # copies (gpsimd cannot access PSUM)
                        nc.scalar.copy(
                            wh_all[:, jt0 : jt0 + 2, :], ps_p[:, :, 0:F_OUT]
                        )
                        nc.scalar.activation(
                            out=et_cols[:, jt0 : jt0 + 2],
                            in_=ps_p[:, :, F_OUT : F_OUT + 1],
                            func=Act.Exp,
                        )
                        nc.vector.tensor_scalar(
                            out=ts_cols[:, jt0 : jt0 + 2],
                            in0=ps_p[:, :, F_OUT : F_OUT + 1],
                            scalar1=-LNS, scalar2=None, op0=Alu.add,
                        )

                z_early = {}

                def emit_z(t, zp):
                    # dve-z P-pairs fuse the clamp: (es*et) max c in one 4x op
                    fuse = t in P_PAIRS and _ZSRC[t] == "dve"
                    for v in range(2):
                        jt = 2 * t + v
                        if _ZSRC[t] == "act":
                            nc.scalar.activation(
                                out=zp[:, v, :], in_=s_bc, func=Act.Exp,
                                bias=ts_cols[:, jt : jt + 1],
                            )
                        elif fuse:
                            nc.vector.tensor_scalar(
                                out=zp[:, v, :], in0=es_bc,
                                scalar1=et_cols[:, jt : jt + 1],
                                scalar2=SCALE, op0=Alu.mult, op1=Alu.max,
                            )
                        else:
                            nc.vector.tensor_scalar(
                                out=zp[:, v, :], in0=es_bc,
                                scalar1=et_cols[:, jt : jt + 1],
                                scalar2=None, op0=Alu.mult,
                            )

                def emit_pair(t):
                    at = at_tiles[t // 4]
                    s0 = 2 * (t % 4)
                    atsl = at[:, s0 : s0 + 2, :]
                    first, last = t == 0, t == NP - 1
                    fused = t in P_PAIRS and _ZSRC[t] == "dve"
                    if t in z_early:
                        zp = z_early[t]
                    elif not fused:
                        zp = zz_pool.tile([128, 2, ROWS], BF16)
                        emit_z(t, zp)
                    if t in P_PAIRS:
                        # clamp fused into the z op for dve-z pairs; Pool
                        # applies the mask multiply (carrier {0, c})
                        if _ZSRC[t] == "dve" and t in z_early:
                            rt = z_early[t]
                        elif _ZSRC[t] == "dve":
                            rt = pp_pool.tile([128, 2, ROWS], BF16, tag="rt")
                            for v in range(2):
                                jt = 2 * t + v
                                nc.vector.tensor_scalar(
                                    out=rt[:, v, :], in0=es_bc,
                                    scalar1=et_cols[:, jt : jt + 1],
                                    scalar2=SCALE, op0=Alu.mult, op1=Alu.max,
                                )
                        else:
                            rt = pp_pool.tile([128, 2, ROWS], BF16, tag="rt")
                            nc.vector.tensor_scalar(
                                out=rt, in0=zp, scalar1=SCALE, scalar2=None,
                                op0=Alu.max,
                            )
                        pp = pp_pool.tile([128, 2, ROWS], FP8, tag="p8")
                        nc.gpsimd.tensor_tensor(
                            out=pp, in0=rt, in1=atsl, op=Alu.mult
                        )
                    else:
                        pp = pp_pool.tile([128, 2, ROWS], FP8, tag="p8")
                        nc.vector.scalar_tensor_tensor(
                            out=pp, in0=zp, scalar=SCALE,
                            in1=atsl, op0=Alu.max, op1=Alu.mult,
                        )
                    if True:
                        for h in range(2):
                            sl = slice(512 * h, 512 * (h + 1))
                            nc.tensor.matmul(
                                ps_oT[:, sl],
                                wh_all[:, 2 * t : 2 * t + 2, :],
                                pp[:, :, sl], start=first, stop=last,
                                perf_mode=DR, skip_group_check=True,
                            )
                            nc.tensor.matmul(
                                ps_d[:, sl], ones_c, pp[:, :, sl],
                                start=first, stop=last,
                                perf_mode=DR, skip_group_check=True,
                            )

                # software-pipelined emission: setup(g+1) ahead of pairs(g);
                # z ops for group 0's pairs go ahead of group 1's copies
                emit_setup(0)
                for _te in range(4):
                    zpe = zz_pool.tile([128, 2, ROWS], BF16, tag=f"zpe{_te}", bufs=1, name=f"zpe{_te}")
                    z_early[_te] = zpe
                    emit_z(_te, zpe)
                for g in range(NG):
                    if g + 1 < NG:
                        emit_setup(g + 1)
                    for t in range(4 * g, 4 * g + 4):
                        emit_pair(t)

            # ---------------- epilogue ----------------
            with tc.tile_pool(name="psE", bufs=2, space="PSUM") as psE:
                den_row = epi.tile([1, ROWS], FP32, tag="den")
                nc.scalar.copy(den_row, ps_d[0:1, :])
                ps_dc = psE.tile([128, 8], FP32, tag="dc")
                for q in range(8):
                    nc.tensor.matmul(
                        ps_dc[:, q : q + 1],
                        den_row[:, 128 * q : 128 * (q + 1)], ones1,
                        start=True, stop=True, skip_group_check=True,
                    )
                rec_cols = epi.tile([128, 8], FP32, tag="rec")
                nc.vector.reciprocal(rec_cols, ps_dc)
                num_sb = epi.tile([128, ROWS], FP32, tag="num")
                for q in range(8):
                    qs = slice(128 * q, 128 * (q + 1))
                    nc.vector.tensor_copy(num_sb[:, qs], ps_oT[:, qs])
                of_all = epi.tile([128, 8, F_OUT], FP32, tag="of")
                for q in range(8):
                    ps_f = psE.tile([128, 128], FP32, tag="f")
                    nc.tensor.transpose(
                        ps_f, num_sb[:, 128 * q : 128 * (q + 1)], idn
                    )
                    m0 = epi.tile([128, 128], FP32, tag=f"m0_{q % 2}")
                    nc.vector.tensor_scalar(
                        out=m0, in0=ps_f, scalar1=rec_cols[:, q : q + 1],
                        scalar2=0.0, op0=Alu.mult, op1=Alu.min,
                    )
                    r0 = epi.tile([128, 128], FP32, tag=f"r0_{q % 2}")
                    nc.scalar.activation(
                        out=r0, in_=ps_f, func=Act.Relu,
                        scale=rec_cols[:, q : q + 1],
                    )
                    e0 = epi.tile([128, 128], FP32, tag=f"e0_{q % 2}")
                    nc.scalar.activation(out=e0, in_=m0, func=Act.Exp)
                    nc.vector.scalar_tensor_tensor(
                        out=of_all[:, q, :], in0=e0, scalar=-1.0, in1=r0,
                        op0=Alu.add, op1=Alu.add,
                    )
                    if q == 3:
                        nc.sync.dma_start(
                            out=out_d[0 : 4 * 128, :].rearrange(
                                "(q p) f -> p q f", p=128
                            ),
                            in_=of_all[:, 0:4, :],
                        )
                nc.sync.dma_start(
                    out=out_d[4 * 128 : 8 * 128, :].rearrange(
                        "(q p) f -> p q f", p=128
                    ),
                    in_=of_all[:, 4:8, :],
                )


def kernel(X, A, weight, a, _trace=False, _tmpdir=None):
    X = np.ascontiguousarray(np.asarray(X, dtype=np.float32))
    A = np.ascontiguousarray(np.asarray(A, dtype=np.int32))
    weight = np.ascontiguousarray(np.asarray(weight, dtype=np.float32))
    a = np.ascontiguousarray(np.asarray(a, dtype=np.float32))

    if "nc" not in _cache:
        _cache["nc"] = _build()
    nc = _cache["nc"]

    bf16 = ml_dtypes.bfloat16
    fp8 = ml_dtypes.float8_e4m3

    Xbf = X.astype(bf16)
    # XTg[g, k, p, c] = X[1024 g + c, 128 k + p]
    XTg_base = np.ascontiguousarray(
        Xbf.reshape(NG, ROWS, KC, 128).transpose(0, 2, 3, 1)
    )
    w_bf = weight.astype(bf16)
    ident = np.eye(128, dtype=np.float32)

    in_maps = []
    for c in range(N_CORES):
        i0 = c * ROWS
        # rotate groups so group 0 is this core's own rows; AT8 rows
        # follow the same j-permutation
        perm = [(c + g) % NG for g in range(NG)]
        XTg = np.ascontiguousarray(XTg_base[perm])
        Asl = (A[i0 : i0 + ROWS].astype(np.float32).T * SCALE).astype(fp8)
        AT8 = np.ascontiguousarray(
            Asl.reshape(NG, ROWS, ROWS)[perm].reshape(N, ROWS)
        )
        in_maps.append(
            {
                "XTg": XTg,
                "AT8": AT8,
                "w_in": w_bf,
                "a_vec": a,
                "ident": ident,
            }
        )

    res = run_bass_kernel_spmd(
        nc, in_maps, core_ids=list(range(N_CORES)), trace=_trace, tmpdir=_tmpdir
    )
    out = np.concatenate([res.results[c]["out"] for c in range(N_CORES)], axis=0)
    if _trace:
        kernel._last_results = res
    return out


# revision 110
# speedup vs baseline: 1.0527x; 1.0028x over previous
"""GAT layer (nn_GATLayer) on 8 Trainium2 NeuronCores.

Math (reference):
    Wh = X @ weight                      [N, F]
    s  = Wh @ a[:F];  t = Wh @ a[F:]     [N, 1]
    e  = relu(s_i + t_j)                 [N, N]
    att = softmax(where(A > 0, e, -9e15), axis=1)
    out = elu(att @ Wh)

Kernel formulation (shift-free softmax, exact up to fp rounding):
    p_ij  = A_ij * max(exp(s_i) * exp(t_j), 1)   (exp(relu(x)) = max(exp(x), 1))
    out_i = elu((p_i: @ Wh) / sum_j p_ij)
A global scale c (=1/4) keeps all fp8 operands in e4m3 normal range:
the host mask carries {0, c}, z' = exp(s - ln(1/c)) * exp(t), and every
psum contribution is uniformly c^2-scaled, which cancels in num/den.

Sharding: 1D row partition across 8 cores (1024 rows each). Host-side
prep is layout/dtype only: X^T (bf16, grouped, rotated so group 0 is
own rows), A^T slab per core (fp8 {0, c}, same group rotation), weight
bf16. All model math (Wh, s, t, exp, softmax, aggregation, ELU) runs
on device.

Per-core dataflow, all in [j (partition), i (free)] orientation:
  - setup: w_all[k] = [weight_k | w*a2 | w*a1]; per j-tile pair one PE
    pass gives [Wh | t] (fp8 Wh); s from group-0 X^T; es/s broadcast
    rows via K=1 PE outer products; t transposed to a row for exp.
  - main loop over 32 j-tile pairs, three z sources balanced across
    engines (PE rank-1 outer product into bf16 psum / DVE 4x ptr-mult
    / ACT exp with bias ptr); two mask paths:
      D-pairs: p8 = (c max z) * mask in one DVE op -> 2 DoubleRow
        matmuls (numerator [128 f, 1024 i], denominator [1, 1024]);
      P-pairs: mask-term DR matmuls consume the raw fp8 mask, the
        relu-term r8 = Pool tensor_tensor mult of relu(z - c).
  - epilogue: den -> columns via K=1 matmuls, reciprocal, PE
    transposes to natural [i, f], fused scale+ELU, one output DMA.
"""

import numpy as np
import ml_dtypes

import concourse.bass as bass
import concourse.bacc as bacc
import concourse.mybir as mybir
import concourse.tile as tile
from concourse.bass_utils import run_bass_kernel_spmd

N = 8192
F_IN = 512
F_OUT = 128
N_CORES = 8
ROWS = N // N_CORES          # 1024 rows per core
NT = N // 128                # 64 j tiles
NP = NT // 2                 # 32 j tile pairs
KC = F_IN // 128             # 4 f_in chunks
NG = 8                       # XT groups (8 j-tiles each)

SCALE = 0.25                 # global fp8 range scale c
LNS = float(np.log(1.0 / SCALE))

FP32 = mybir.dt.float32
BF16 = mybir.dt.bfloat16
FP8 = mybir.dt.float8e4
Alu = mybir.AluOpType
Act = mybir.ActivationFunctionType
DR = mybir.MatmulPerfMode.DoubleRow

# --- engine lane tables (tuned against the TimelineSim cost model) ---
# P_PAIRS: pairs routed through the relu decomposition (Pool mask-mult)
P_PAIRS = frozenset({1, 3, 5, 6, 9, 11, 13, 14, 17, 19, 21, 22, 25, 26})
# z source per pair: 'pe' (rank-1 matmul into psum, D-pairs only),
# 'act' (exp), 'dve' (ptr-mult)
_ZSRC = {}
for _t in range(NP):
    _ZSRC[_t] = "dve" if _t in P_PAIRS else "act"
# relu engine for P-pairs: DVE 4x for most, ACT for some
_RELU_ACT = frozenset({3, 11, 19, 27})

_cache = {}


def _build():
    nc = bacc.Bacc("TRN2", target_bir_lowering=False, debug=False,
                   num_devices=N_CORES)

    XTg = nc.dram_tensor("XTg", [NG, KC, 128, ROWS], BF16, kind="ExternalInput")
    AT8 = nc.dram_tensor("AT8", [N, ROWS], FP8, kind="ExternalInput")
    w_in = nc.dram_tensor("w_in", [F_IN, F_OUT], BF16, kind="ExternalInput")
    a_vec = nc.dram_tensor("a_vec", [2 * F_OUT, 1], FP32, kind="ExternalInput")
    ident = nc.dram_tensor("ident", [128, 128], FP32, kind="ExternalInput")
    out_d = nc.dram_tensor("out", [ROWS, F_OUT], FP32, kind="ExternalOutput")

    with tile.TileContext(nc) as tc:
        _body(nc, tc, XTg, AT8, w_in, a_vec, ident, out_d)

    nc.compile()
    return nc


def _body(nc, tc, XTg, AT8, w_in, a_vec, ident, out_d):
    with (
        tc.tile_pool(name="setup", bufs=1) as setup,
        tc.tile_pool(name="xtg", bufs=2) as xtg_pool,
        tc.tile_pool(name="at", bufs=1) as at_pool,
        tc.tile_pool(name="zz", bufs=5) as zz_pool,
        tc.tile_pool(name="pp", bufs=6) as pp_pool,
        tc.tile_pool(name="epi", bufs=1) as epi,
    ):
        setup_psum = tc.tile_pool(name="psA", bufs=1, space="PSUM")
        psA = setup_psum.__enter__()
        psW_cm = tc.tile_pool(name="psW", bufs=1, space="PSUM")
        psW = psW_cm.__enter__()
        # ---------------- setup: weights ----------------
        w_sb = setup.tile([128, KC, 128], BF16)
        nc.sync.dma_start(
            out=w_sb, in_=w_in.rearrange("(k p) f -> p k f", p=128)
        )
        idn = setup.tile([128, 128], FP32)
        nc.sync.dma_start(out=idn, in_=ident[:, :])
        a_cat = setup.tile([128, 2], BF16)
        nc.gpsimd.dma_start(
            out=a_cat, in_=a_vec.rearrange("(h p) o -> p (h o)", p=128)
        )

        # first XT group doubles as own-rows X^T (host rotates groups)
        xtg0 = xtg_pool.tile([128, KC, ROWS], BF16, tag="g0")
        for k in range(KC):
            nc.sync.dma_start(
                out=xtg0[:, k, :],
                in_=XTg[0].rearrange("k p i -> p k i")[:, k, :],
            )
        idn_bf = setup.tile([128, 128], BF16)
        nc.vector.tensor_copy(idn_bf, idn)

        # w_all[k] = [weight_k | w_t_k | w_s_k]  [128, 130]
        w_all = []
        for k in range(KC):
            wa = setup.tile([128, F_OUT + 2], BF16, tag=f"w_all{k}")
            nc.vector.tensor_copy(wa[:, 0:F_OUT], w_sb[:, k, :])
            ps_wT = psW.tile([128, 128], BF16, tag="wT")
            nc.tensor.transpose(ps_wT, w_sb[:, k, :], idn_bf)
            wT = setup.tile([128, 128], BF16, tag=f"wT{k}")
            nc.vector.tensor_copy(wT, ps_wT)
            ps_a = psW.tile([128, 2], FP32, tag="pa")
            nc.tensor.matmul(ps_a, wT, a_cat, start=True, stop=True)
            # col F_OUT = w_t (a[F:]), col F_OUT+1 = w_s (a[:F])
            nc.vector.tensor_copy(wa[:, F_OUT : F_OUT + 1], ps_a[:, 1:2])
            nc.vector.tensor_copy(wa[:, F_OUT + 1 : F_OUT + 2], ps_a[:, 0:1])
            w_all.append(wa)
        psW_cm.__exit__(None, None, None)


        # ---------------- s (own rows) + broadcast rows ----------------
        ps_s = psA.tile([1, ROWS], FP32, tag="ps_s")
        for h in range(2):
            sl = slice(512 * h, 512 * (h + 1))
            for k in range(KC):
                nc.tensor.matmul(
                    ps_s[:, sl],
                    w_all[k][:, F_OUT + 1 : F_OUT + 2],
                    xtg0[:, k, sl],
                    start=(k == 0), stop=(k == KC - 1),
                    skip_group_check=True,
                )
        nls1 = setup.tile([1, 1], FP32, tag="nls1")
        nc.vector.memset(nls1, -LNS)
        es_row = setup.tile([1, ROWS], BF16)
        s_row = setup.tile([1, ROWS], BF16)
        for h in range(2):
            sl = slice(512 * h, 512 * (h + 1))
            nc.scalar.activation(
                out=es_row[:, sl], in_=ps_s[:, sl], func=Act.Exp, bias=nls1
            )
            nc.vector.tensor_copy(s_row[:, sl], ps_s[:, sl])

        ones_r = setup.tile([1, 128], BF16)
        nc.vector.memset(ones_r, 1.0)
        es_bc = setup.tile([128, ROWS], BF16)
        s_bc = setup.tile([128, ROWS], BF16)
        for h in range(2):
            sl = slice(512 * h, 512 * (h + 1))
            ps_b = psA.tile([128, 512], FP32, tag="bc", bufs=2)
            nc.tensor.matmul(ps_b, ones_r, es_row[:, sl], start=True, stop=True)
            nc.vector.tensor_copy(es_bc[:, sl], ps_b)
            ps_b2 = psA.tile([128, 512], FP32, tag="bc", bufs=2)
            nc.tensor.matmul(ps_b2, ones_r, s_row[:, sl], start=True, stop=True)
            nc.scalar.copy(s_bc[:, sl], ps_b2)

        ones_c = setup.tile([128, 2, 128], FP8)
        nc.vector.memset(ones_c, 1.0)
        ones1 = setup.tile([1, 1], FP32, tag="ones1")
        nc.vector.memset(ones1, 1.0)

        # ---------------- Wh | t for all j tiles ----------------
        wh_all = setup.tile([128, NT, F_OUT], FP8)
        et_cols = setup.tile([128, NT], FP32)
        ts_cols = setup.tile([128, NT], FP32)
        nsc_c = setup.tile([128, 1], FP32, tag="nsc_c")
        nc.vector.memset(nsc_c, -SCALE)
        nsc_ln = setup.tile([128, 1], FP32, tag="nsc_ln")
        nc.vector.memset(nsc_ln, -LNS)

        setup_psum.__exit__(None, None, None)

        with (
            tc.tile_pool(name="psO", bufs=1, space="PSUM") as psO,
            tc.tile_pool(name="psD", bufs=1, space="PSUM") as psD,
        ):
            ps_oT = psO.tile([128, ROWS], FP32)
            ps_d = psD.tile([128, ROWS], FP32)

            with tc.tile_pool(name="psS", bufs=4, space="PSUM") as psS:
                at_tiles = []

                def emit_setup(g):
                    if g == 0:
                        xtg = xtg0
                    else:
                        xtg = xtg_pool.tile([128, KC, ROWS], BF16)
                        nc.sync.dma_start(
                            out=xtg, in_=XTg[g].rearrange("k p i -> p k i")
                        )
                    at = at_pool.tile([128, 8, ROWS], FP8, tag=f"at{g}")
                    at_tiles.append(at)
                    nc.sync.dma_start(
                        out=at,
                        in_=AT8[ROWS * g : ROWS * (g + 1), :].rearrange(
                            "(t p) i -> p t i", p=128
                        ),
                    )
                    for q in range(4):
                        jt0 = 8 * g + 2 * q
                        ps_p = psS.tile([128, 2, F_OUT + 2], FP32)
                        for v in range(2):
                            co = 128 * (2 * q + v)
                            for k in range(KC):
                                nc.tensor.matmul(
                                    ps_p[:, v, :],
                                    xtg[:, k, co : co + 128],
                                    w_all[k],
                                    start=(k == 0), stop=(k == KC - 1),
                                    skip_group_check=True,
                                )
                        # copies (gpsimd cannot access PSUM)
                        nc.scalar.copy(
                            wh_all[:, jt0 : jt0 + 2, :], ps_p[:, :, 0:F_OUT]
                        )
                        nc.scalar.activation(
                            out=et_cols[:, jt0 : jt0 + 2],
                            in_=ps_p[:, :, F_OUT : F_OUT + 1],
                            func=Act.Exp,
                        )
                        nc.vector.tensor_scalar(
                            out=ts_cols[:, jt0 : jt0 + 2],
                            in0=ps_p[:, :, F_OUT : F_OUT + 1],
                            scalar1=-LNS, scalar2=None, op0=Alu.add,
                        )

                z_early = {}

                def emit_z(t, zp):
                    # dve-z P-pairs fuse the clamp: (es*et) max c in one 4x op
                    fuse = t in P_PAIRS and _ZSRC[t] == "dve"
                    for v in range(2):
                        jt = 2 * t + v
                        if _ZSRC[t] == "act":
                            nc.scalar.activation(
                                out=zp[:, v, :], in_=s_bc, func=Act.Exp,
                                bias=ts_cols[:, jt : jt + 1],
                            )
                        elif fuse:
                            nc.vector.tensor_scalar(
                                out=zp[:, v, :], in0=es_bc,
                                scalar1=et_cols[:, jt : jt + 1],
                                scalar2=SCALE, op0=Alu.mult, op1=Alu.max,
                            )
                        else:
                            nc.vector.tensor_scalar(
                                out=zp[:, v, :], in0=es_bc,
                                scalar1=et_cols[:, jt : jt + 1],
                                scalar2=None, op0=Alu.mult,
                            )

                def emit_pair(t):
                    at = at_tiles[t // 4]
                    s0 = 2 * (t % 4)
                    atsl = at[:, s0 : s0 + 2, :]
                    first, last = t == 0, t == NP - 1
                    fused = t in P_PAIRS and _ZSRC[t] == "dve"
                    if t in z_early:
                        zp = z_early[t]
                    elif not fused:
                        zp = zz_pool.tile([128, 2, ROWS], BF16)
                        emit_z(t, zp)
                    if t in P_PAIRS:
                        # clamp fused into the z op for dve-z pairs; Pool
                        # applies the mask multiply (carrier {0, c})
                        if _ZSRC[t] == "dve" and t in z_early:
                            rt = z_early[t]
                        elif _ZSRC[t] == "dve":
                            rt = pp_pool.tile([128, 2, ROWS], BF16, tag="rt")
                            for v in range(2):
                                jt = 2 * t + v
                                nc.vector.tensor_scalar(
                                    out=rt[:, v, :], in0=es_bc,
                                    scalar1=et_cols[:, jt : jt + 1],
                                    scalar2=SCALE, op0=Alu.mult, op1=Alu.max,
                                )
                        else:
                            rt = pp_pool.tile([128, 2, ROWS], BF16, tag="rt")
                            nc.vector.tensor_scalar(
                                out=rt, in0=zp, scalar1=SCALE, scalar2=None,
                                op0=Alu.max,
                            )
                        pp = pp_pool.tile([128, 2, ROWS], FP8, tag="p8")
                        nc.gpsimd.tensor_tensor(
                            out=pp, in0=rt, in1=atsl, op=Alu.mult
                        )
                    else:
                        pp = pp_pool.tile([128, 2, ROWS], FP8, tag="p8")
                        nc.vector.scalar_tensor_tensor(
                            out=pp, in0=zp, scalar=SCALE,
                            in1=atsl, op0=Alu.max, op1=Alu.mult,
                        )
                    if True:
                        for h in range(2):
                            sl = slice(512 * h, 512 * (h + 1))
                            nc.tensor.matmul(
                                ps_oT[:, sl],
                                wh_all[:, 2 * t : 2 * t + 2, :],
                                pp[:, :, sl], start=first, stop=last,
                                perf_mode=DR, skip_group_check=True,
                            )
                            nc.tensor.matmul(
                                ps_d[:, sl], ones_c, pp[:, :, sl],
                                start=first, stop=last,
                                perf_mode=DR, skip_group_check=True,
                            )

                # software-pipelined emission: setup(g+1) ahead of pairs(g);
                # z ops for group 0's pairs go ahead of group 1's copies
                emit_setup(0)
                for _te in range(4):
                    zpe = zz_pool.tile([128, 2, ROWS], BF16, tag=f"zpe{_te}", bufs=1, name=f"zpe{_te}")
                    z_early[_te] = zpe
                    emit_z(_te, zpe)
                for g in range(NG):
                    if g + 1 < NG:
                        emit_setup(g + 1)
                    for t in range(4 * g, 4 * g + 4):
                        emit_pair(t)

            # ---------------- epilogue ----------------
            with tc.tile_pool(name="psE", bufs=2, space="PSUM") as psE:
                den_row = epi.tile([1, ROWS], FP32, tag="den")
                nc.scalar.copy(den_row, ps_d[0:1, :])
                ps_dc = psE.tile([128, 8], FP32, tag="dc")
                for q in range(8):
                    nc.tensor.matmul(
                        ps_dc[:, q : q + 1],
                        den_row[:, 128 * q : 128 * (q + 1)], ones1,
                        start=True, stop=True, skip_group_check=True,
                    )
                rec_cols = epi.tile([128, 8], FP32, tag="rec")
                nc.vector.reciprocal(rec_cols, ps_dc)
                num_sb = epi.tile([128, ROWS], FP32, tag="num")
                for q in range(8):
                    qs = slice(128 * q, 128 * (q + 1))
                    nc.vector.tensor_copy(num_sb[:, qs], ps_oT[:, qs])
                of_all = epi.tile([128, 8, F_OUT], FP32, tag="of")
                for q in range(8):
                    ps_f = psE.tile([128, 128], FP32, tag="f")
                    nc.tensor.transpose(
                        ps_f, num_sb[:, 128 * q : 128 * (q + 1)], idn
                    )
                    m0 = epi.tile([128, 128], FP32, tag=f"m0_{q % 2}")
                    nc.vector.tensor_scalar(
                        out=m0, in0=ps_f, scalar1=rec_cols[:, q : q + 1],
                        scalar2=0.0, op0=Alu.mult, op1=Alu.min,
                    )
                    r0 = epi.tile([128, 128], FP32, tag=f"r0_{q % 2}")
                    nc.scalar.activation(
                        out=r0, in_=ps_f, func=Act.Relu,
                        scale=rec_cols[:, q : q + 1],
                    )
                    e0 = epi.tile([128, 128], FP32, tag=f"e0_{q % 2}")
                    nc.scalar.activation(out=e0, in_=m0, func=Act.Exp)
                    nc.vector.scalar_tensor_tensor(
                        out=of_all[:, q, :], in0=e0, scalar=-1.0, in1=r0,
                        op0=Alu.add, op1=Alu.add,
                    )
                    if q == 3:
                        nc.sync.dma_start(
                            out=out_d[0 : 4 * 128, :].rearrange(
                                "(q p) f -> p q f", p=128
                            ),
                            in_=of_all[:, 0:4, :],
                        )
                nc.sync.dma_start(
                    out=out_d[4 * 128 : 8 * 128, :].rearrange(
                        "(q p) f -> p q f", p=128
                    ),
                    in_=of_all[:, 4:8, :],
                )


def kernel(X, A, weight, a, _trace=False, _tmpdir=None):
    X = np.ascontiguousarray(np.asarray(X, dtype=np.float32))
    A = np.ascontiguousarray(np.asarray(A, dtype=np.int32))
    weight = np.ascontiguousarray(np.asarray(weight, dtype=np.float32))
    a = np.ascontiguousarray(np.asarray(a, dtype=np.float32))

    if "nc" not in _cache:
        _cache["nc"] = _build()
    nc = _cache["nc"]

    bf16 = ml_dtypes.bfloat16
    fp8 = ml_dtypes.float8_e4m3

    Xbf = X.astype(bf16)
    # XTg[g, k, p, c] = X[1024 g + c, 128 k + p]
    XTg_base = np.ascontiguousarray(
        Xbf.reshape(NG, ROWS, KC, 128).transpose(0, 2, 3, 1)
    )
    w_bf = weight.astype(bf16)
    ident = np.eye(128, dtype=np.float32)

    in_maps = []
    for c in range(N_CORES):
        i0 = c * ROWS
        # rotate groups so group 0 is this core's own rows; AT8 rows
        # follow the same j-permutation
        perm = [(c + g) % NG for g in range(NG)]
        XTg = np.ascontiguousarray(XTg_base[perm])
        Asl = (A[i0 : i0 + ROWS].astype(np.float32).T * SCALE).astype(fp8)
        AT8 = np.ascontiguousarray(
            Asl.reshape(NG, ROWS, ROWS)[perm].reshape(N, ROWS)
        )
        in_maps.append(
            {
                "XTg": XTg,
                "AT8": AT8,
                "w_in": w_bf,
                "a_vec": a,
                "ident": ident,
            }
        )

    res = run_bass_kernel_spmd(
        nc, in_maps, core_ids=list(range(N_CORES)), trace=_trace, tmpdir=_tmpdir
    )
    out = np.concatenate([res.results[c]["out"] for c in range(N_CORES)], axis=0)
    if _trace:
        kernel._last_results = res
    return out
